# revision 13
# baseline (speedup 1.0000x reference)
"""Trainium2 Bass kernel for CharModel ragged segment-mean + pos embedding.

Computation (per sample):
  out[j, :] = mean(feats[start_j:end_j, :]) * valid_j + pos_table[pos_j]
where the ragged segments are given by sorted word start offsets.

Strategy (fp16 data path; harness gate is rel_err < 2e-2 and fp16 lands
at ~7e-4, so no hi/lo compensation is needed):
  - Host precomputes per-char metadata: word_id[c] (which word each char
    belongs to, -1 for padding chars) and wrec[c] = 1/len(word(c)).
  - Device builds a one-hot matrix M[c, j] = (word_id[c]==j) with one
    fused DVE tensor_scalar op per 128-char tile, then the PE computes
    sum[W, D] = M.T @ feats directly in PSUM.  The pos embedding is
    added by accumulating (onehot(pos)*len).T @ pos_table into the same
    PSUM (both operands zero-padded to 128 contraction rows so the PE
    never switches tile quadrants), so the final 1/len multiply leaves
    mean + pos_table[pos].
  - feats ship pre-transposed in DRAM as [128, NT*D] fp16 (char c lives
    at partition c%128, tile c//128) so each per-sample feats DMA is
    per-partition contiguous: 128 descriptors instead of 128*NT, which
    cuts the HWDGE DIRECT2D issue cost ~4x.  The output is staged and
    written the same way ([128, NG*D] fp16, partition-contiguous) and
    un-swizzled on the host.
  - All DMAs ride the sync (HWDGE) queue with the metadata packs FIRST:
    SDMA rings drain in enqueue order, so const-after-feats would stall
    the warm-up + one-hot builds ~6us behind sample-0 feats data.
  - Data parallel over batch: 8 NeuronCores x 4 samples each, one shared
    SPMD program, per-core input maps.

Walrus ISA wait-slot limits dealt with throughout: matmul (S3_LW),
tensor_scalar (S3D3_TS) and DMA (PSEUDO_DMA_DIRECT2D) instructions can
carry only ONE semaphore wait each.  Hence:
  - the fp16 metadata (iota row + word-id scalars) ships in ONE packed
    tensor; the fp32 copy-scale reciprocals in a tiny second pack that a
    Scalar probe and a Vector probe observe once, so the PSUM->SBUF
    copies carry only their PE wait;
  - a per-sample 1x1 "gate" matmul (forced first in PE order via
    add_dep_helper, writing a dedicated never-overlapping PSUM region)
    alone carries the DVE wait for the lhsT builds and, transitively, the
    previous sample's PSUM bank releases;
  - feats tiles and output staging tiles get enough pool bufs that no
    slot is ever reused (no WAR waits on DMAs/copies);
  - two output DMAs per sample (SWDGE, each preceded by a Pool probe of
    the DVE-written half so the DMA carries only the ACT copy wait).
"""

import sys

if "/opt/trn_rl_repo" not in sys.path:
    sys.path.insert(0, "/opt/trn_rl_repo")

import numpy as np

import bass_rust
import concourse.bass as bass
import concourse.mybir as mybir
from concourse.tile import TileContext
from concourse.tile_sem_assignment import N_PROCS


class ChunkedDrainTileContext(TileContext):
    """TileContext whose kernel-tail drain is split into several drain
    instructions with one sem wait each (the CTRL_NO ISA struct rejects
    multi-wait drains here)."""

    DRAIN_CHUNK = 1

    def _drain_and_barrier(self, tick_clock, wait_clock):
        gc = tick_clock.global_clock
        ticks = [gc.peek_next(i) - 1 for i in range(N_PROCS)]
        active = [i for i, t in enumerate(ticks) if t > 0]
        for i in range(0, len(active), self.DRAIN_CHUNK):
            chunk = set(active[i : i + self.DRAIN_CHUNK])
            part = [ticks[j] if j in chunk else 0 for j in range(N_PROCS)]
            d = self.nc.sync.drain()
            wait_clock.add_sem_waits(
                d.ins, bass_rust.ScopedClock({None: bass_rust.VectorClock(part)})
            )
        self.nc.all_engine_barrier()
        assert self.sems is not None
        popped = self.nc._tile_sem_poison_stack.pop()
        assert popped is self._sem_poison
        self.nc.clear_and_free_semaphores(list(self.sems.allocated().values()))
        self.nc.all_engine_barrier()

B, S, D, W, NPOS = 32, 1024, 512 + 256, 512, 32  # D=768
N_CORES = 8
SPC = B // N_CORES  # samples per core
NT = S // 128  # char tiles per sample
NG = W // 128  # word groups per sample
CHUNKS = ((0, 384), (384, 384))  # D split for PSUM bank limit
F32 = mybir.dt.float32
F16 = mybir.dt.float16

# fp16 pos pack: only the 32 live rows ship over DMA; the zero rows
# 32:128 are memset on the DVE (shipping them would double the pack's
# critical-path DMA time)
CPB_POH = 0  # [32, SPC*W]: sample s cols s*W:(s+1)*W = onehot(pos)*len
CPB_TAB = SPC * W  # [32, D]: fp16(pos_table)
CPB_W = SPC * W + D

# fp32 pack (copy scales + word-id scalars; the is_equal build with f32
# stream + f32 scalar + fp16 out is the HW-proven config).  The iota row
# is generated on-device by the Pool engine, keeping this pack tiny so
# the one-hot builds start as early as possible.
CPR_REC = 0  # [128, NG*SPC]: per sample s: 1/len per word group
CPR_META = NG * SPC  # [128, NT*SPC]: per sample s: word-id per char
CPR_W = NG * SPC + NT * SPC


def _build_program(sched):
    """sched[s][g] = tuple of char-tile indices whose chars can touch word
    group g of slot-s samples on ANY core (union schedule; the one-hot
    lhsT zeroes contributions from tiles/words not actually present on a
    given core).  Matmuls for (g, t) pairs outside the schedule multiply
    all-zero one-hot slices and are skipped entirely."""
    nc = bass.Bass()
    # feats pre-transposed on host: char c of sample s lives at
    # [s, c % 128, (c // 128) * D : (c // 128 + 1) * D].
    feats = nc.declare_dram_parameter("feats", [SPC, 128, NT * D], F16, False)
    constpb = nc.declare_dram_parameter("constpb", [64, CPB_W], F16, False)
    constpr = nc.declare_dram_parameter("constpr", [128, CPR_W], F32, False)
    # output staged the same way: word w of sample s at
    # [s, w % 128, (w // 128) * D : ...]; host un-swizzles.
    out = nc.declare_dram_parameter("out", [SPC, 128, NG * D], F16, True)

    dep = lambda a, b, why: bass_rust.add_dep_helper(
        a.ins, b.ins, sync=False, reason=why
    )

    # per-sample used tile range (contiguous from 0 by construction)
    tmax = [max(t for g in range(NG) for t in sched[s][g]) for s in range(SPC)]
    with ChunkedDrainTileContext(nc) as tc:
        with (
            tc.tile_pool(name="const", bufs=1) as cpool,
            tc.tile_pool(name="feat", bufs=SPC) as fpool,
            tc.tile_pool(name="lhs", bufs=sum(t + 1 for t in tmax)) as lpool,
            tc.tile_pool(name="outsb", bufs=SPC) as opool,
            tc.tile_pool(name="psum", bufs=2 * NG - 1, space="PSUM") as ppool,
            tc.tile_pool(name="gatep", bufs=1, space="PSUM") as gpool,
        ):
            cpb = cpool.tile([128, CPB_W], F16)
            # iota row 0..W-1 on every partition, generated on-device
            # (fp32 is exact for these integers)
            iota_t = cpool.tile([128, W], F32)
            nc.gpsimd.iota(
                iota_t[:, :],
                pattern=[[1, W]],
                base=0,
                channel_multiplier=0,
                allow_small_or_imprecise_dtypes=True,
            )
            iota_f = iota_t[:, :]
            cpr = cpool.tile([128, CPR_W], F32)
            cpr_dma = nc.sync.dma_start(out=cpr[:, :], in_=constpr[:, :])
            # ACT probe: observe the f32-pack DMA on the Scalar engine once,
            # so the ACT PSUM->SBUF copies carry only their PE wait.  (The
            # DVE observes it via the first one-hot build.)  The DVE probe
            # observes the Pool iota tick so the builds carry only the
            # f32-pack wait.
            act_probe = cpool.tile([1, 2], F32)
            nc.scalar.copy(act_probe[0:1, 0:1], cpr[0:1, 0:1])
            nc.vector.tensor_copy(act_probe[0:1, 1:2], iota_t[0:1, 0:1])
            pl_probe = cpool.tile([1, 2 * SPC], F16)
            # cpb rows 64:128 zeroed on the Pool engine AFTER the iota (the
            # DMA ships rows 0:64, with 32:64 zero from the host); a tiny
            # "pgate" matmul below carries the memset tick for the pos
            # matmuls.
            nc.gpsimd.memset(cpb[64:128, :], 0.0)
            # Combined gate + warm-up PSUM bank: gates write cols 0:16,
            # the warm-up accumulates into cols 16:512 of row 0.
            gp = gpool.tile([128, 512], F32)
            gate_t = gp[:, 0:32]

            # PE warm-up: fat fp32 matmuls reading the on-device iota tile
            # run during the DMA ramp and trip the HAM clock gate before
            # the real matmuls start.  Without this the PE sometimes stays
            # at 1.2GHz for the whole kernel.
            for wi in range(5):
                nc.tensor.matmul(
                    gp[0:1, 32:224],
                    iota_t[:, 0:1],
                    iota_t[:, 0:192],
                    start=(wi == 0),
                    stop=(wi == 4),
                    skip_group_check=True,
                )
            # pgate: carries the Pool memset tick; every pos matmul deps on
            # it so none carries a second wait besides the cpb DMA sem.
            pgate = nc.tensor.matmul(
                gate_t[0:1, 29:30],
                cpb[64:65, 0:1],
                cpb[64:65, 0:1],
                start=True,
                stop=True,
                skip_group_check=True,
            )
            # feats DMAs: sample 0 in two pieces (so group-0/1 matmuls can
            # start as soon as their tiles land), later samples whole:
            # 7 sync-queue DMAs total, within the 8 HWDGE queues.
            fts = []
            for s in range(SPC):
                ft = fpool.tile([128, NT * D], F16, tag="ft", name=f"ft_{s}")
                fts.append(ft)
            # Input phasing: {cpr, s0} stream immediately; cpb starts once
            # the tiny f32 pack is in; {s1,s2,s3} start after the pos pack.
            # Each arrives just ahead of its consumer while the
            # head-critical packs get most of the bandwidth.
            ntile0 = tmax[0] + 1
            s0_dma = nc.sync.dma_start(
                out=fts[0][:, : ntile0 * D], in_=feats[0, :, : ntile0 * D]
            )
            cpb_dma = nc.sync.dma_start(out=cpb[0:64, :], in_=constpb[:, :])
            bass_rust.add_dep_helper(
                cpb_dma.ins, cpr_dma.ins, reason="pos pack after f32 pack"
            )
            for s in range(1, SPC):
                fd = nc.sync.dma_start(
                    out=fts[s][:, : (tmax[s] + 1) * D],
                    in_=feats[s, :, : (tmax[s] + 1) * D],
                )
                bass_rust.add_dep_helper(
                    fd.ins, s0_dma.ins, reason="feats tail after sample 0"
                )

            def emit_builds(s):
                d = {}
                for t in range(tmax[s] + 1):
                    lh = lpool.tile([128, W], F16, tag="lh", name=f"lh_{s}_{t}")
                    wcol = CPR_META + NT * s
                    nc.vector.tensor_scalar(
                        lh[:, :],
                        iota_f,
                        cpr[:, wcol + t : wcol + t + 1],
                        None,
                        op0=mybir.AluOpType.is_equal,
                    )
                    d[t] = lh
                return d

            lhs_by_s = {0: emit_builds(0)}
            prev_ob = None  # previous sample's output staging buffer
            for s in range(SPC):
                ft = fts[s]
                last_dve_copy = None
                last_act_copy = None
                ntile = tmax[s] + 1
                lhs = lhs_by_s[s]

                # Gate A: 1x1x1 matmul reading a late lhsT build; forced
                # first in PE order so it alone carries the DVE wait for
                # this sample's builds.  For sample 0 (head-critical) it
                # only covers the builds groups 0/1 consume; a second gate
                # at the halfway point covers the rest.
                if s == 0:
                    tA = max(t for g in (0, 1) for t in sched[s][g])
                else:
                    tA = ntile - 1
                gate = nc.tensor.matmul(
                    gate_t[0:1, s : s + 1],
                    lhs[tA][0:1, 0:1],
                    lhs[tA][0:1, 0:1],
                    start=True,
                    stop=True,
                    skip_group_check=True,
                )
                # Early bank gates: cover the previous sample's PSUM bank
                # releases that this sample's FIRST half (units 0-3, prev
                # bufs 1-4) actually needs: the ACT copy chain through unit
                # 5 and the DVE copy chain through unit 6.  The last copies
                # of the previous sample are NOT waited on here, so the PE
                # does not stall at sample boundaries.
                if prev_ob is not None:
                    bgate = nc.tensor.matmul(
                        gate_t[0:1, SPC + s : SPC + s + 1],
                        prev_ob[0:1, 2 * D + 384 : 2 * D + 385],
                        prev_ob[0:1, 2 * D + 384 : 2 * D + 385],
                        start=True,
                        stop=True,
                        skip_group_check=True,
                    )
                    bgate2 = nc.tensor.matmul(
                        gate_t[0:1, 2 * SPC + s : 2 * SPC + s + 1],
                        prev_ob[0:1, 3 * D : 3 * D + 1],
                        prev_ob[0:1, 3 * D : 3 * D + 1],
                        start=True,
                        stop=True,
                        skip_group_check=True,
                    )
                else:
                    bgate = bgate2 = None

                ob = opool.tile([128, NG * D], F16, tag="ob", name=f"ob_{s}")
                gate2 = lgate1 = lgate2 = None
                for g in range(NG):
                    if g == 2:
                        # Emit the NEXT sample's one-hot builds here: they
                        # slot into the DVE queue between this sample's
                        # unit-2 and unit-4 copies, so they never sit behind
                        # a copy that is blocked on a late PSUM (DVE
                        # head-of-line), and the next gateA finds them done.
                        if s + 1 < SPC:
                            lhs_by_s[s + 1] = emit_builds(s + 1)
                        # Late gates before the second half (units 4-7, which
                        # reuse the banks of prev units 5-7 and own unit 0):
                        # cover the previous sample's LAST ACT copy and this
                        # sample's first DVE copy.  For sample 0 a second
                        # build gate covers the groups-2/3 one-hot builds.
                        if s == 0:
                            gate2 = nc.tensor.matmul(
                                gate_t[0:1, 28:29],
                                lhs[ntile - 1][0:1, 0:1],
                                lhs[ntile - 1][0:1, 0:1],
                                start=True,
                                stop=True,
                                skip_group_check=True,
                            )
                        if prev_ob is not None:
                            lgate1 = nc.tensor.matmul(
                                gate_t[0:1, 3 * SPC + s : 3 * SPC + s + 1],
                                prev_ob[0:1, NG * D - 1 : NG * D],
                                prev_ob[0:1, NG * D - 1 : NG * D],
                                start=True,
                                stop=True,
                                skip_group_check=True,
                            )
                        lgate2 = nc.tensor.matmul(
                            gate_t[0:1, 4 * SPC + s : 4 * SPC + s + 1],
                            ob[0:1, 0:1],
                            ob[0:1, 0:1],
                            start=True,
                            stop=True,
                            skip_group_check=True,
                        )
                    tiles_g = sched[s][g]
                    pss = [
                        ppool.tile([128, cn], F32, tag="ps", name=f"ps_{s}_{g}_{ci}")
                        for ci, (c0, cn) in enumerate(CHUNKS)
                    ]

                    def _mm_deps(mm):
                        dep(mm, gate, "matmuls after sample gate")
                        if gate2 is not None:
                            dep(mm, gate2, "matmuls after late build gate")
                        if g < 2:
                            if bgate is not None:
                                dep(mm, bgate, "matmuls after early bank gate")
                                dep(mm, bgate2, "matmuls after early bank gate2")
                        else:
                            if lgate1 is not None:
                                dep(mm, lgate1, "matmuls after late bank gate")
                            dep(mm, lgate2, "matmuls after late bank gate2")

                    # chunk-inner order: the two matmuls of a (g, t) pair use
                    # the same stationary weights back to back
                    for k, t in enumerate(tiles_g):
                        for ci, (c0, cn) in enumerate(CHUNKS):
                            mm = nc.tensor.matmul(
                                pss[ci][:, :],
                                lhs[t][:, 128 * g : 128 * (g + 1)],
                                ft[:, t * D + c0 : t * D + c0 + cn],
                                start=(k == 0),
                                stop=False,
                                skip_group_check=True,
                            )
                            _mm_deps(mm)
                    # pos contribution scaled by len (exact in fp16) so the
                    # final 1/len multiply leaves pos_table[pos] exactly;
                    # both operands are zero-padded to 128 rows
                    pcol = CPB_POH + s * W
                    for ci, (c0, cn) in enumerate(CHUNKS):
                        mm = nc.tensor.matmul(
                            pss[ci][:, :],
                            cpb[:, pcol + 128 * g : pcol + 128 * (g + 1)],
                            cpb[:, CPB_TAB + c0 : CPB_TAB + c0 + cn],
                            start=(len(tiles_g) == 0),
                            stop=True,
                            skip_group_check=True,
                        )
                        dep(mm, pgate, "pos matmul after memset gate")
                        _mm_deps(mm)
                    for ci, (c0, cn) in enumerate(CHUNKS):
                        unit = 2 * g + ci
                        recip_ap = cpr[
                            :, CPR_REC + NG * s + g : CPR_REC + NG * s + g + 1
                        ]
                        # last sample: units 0-3 all on ACT so the g1-point
                        # output DMA needs only the chained unit-3 ACT wait
                        use_dve = (unit % 2 == 0) and not (s == SPC - 1 and unit < 4)
                        if use_dve:
                            cp = nc.vector.tensor_scalar(
                                ob[:, g * D + c0 : g * D + c0 + cn],
                                pss[ci][:, :],
                                recip_ap,
                                None,
                                op0=mybir.AluOpType.mult,
                            )
                            if last_dve_copy is not None:
                                dep(cp, last_dve_copy, "DVE copy order")
                            last_dve_copy = cp
                        else:
                            cp = nc.scalar.activation(
                                ob[:, g * D + c0 : g * D + c0 + cn],
                                pss[ci][:, :],
                                mybir.ActivationFunctionType.Copy,
                                scale=recip_ap,
                            )
                            if last_act_copy is not None:
                                dep(cp, last_act_copy, "ACT copy order")
                            last_act_copy = cp
                    # streaming output: sample 0 ships once at the end
                    # (it is never tail-critical); middle samples split at
                    # the g2 point; the last sample ships three pieces so
                    # most of its output flows during its own compute.
                    # Pool probes observe the DVE copy chain so each DMA
                    # carries only the chained ACT wait.
                    last = s == SPC - 1
                    if last and g == 1:
                        # units 0-3 are all ACT copies: the DMA's unit-3
                        # wait covers them by the ACT dep chain
                        nc.gpsimd.dma_start(
                            out=out[s, :, : 2 * D], in_=ob[:, : 2 * D]
                        )
                    if g == 2 and s != 0:
                        lo = 2 * D if last else 0
                        nc.gpsimd.tensor_copy(
                            pl_probe[0:1, 2 * s : 2 * s + 1],
                            ob[0:1, 2 * D : 2 * D + 1],
                        )
                        nc.gpsimd.dma_start(
                            out=out[s, :, lo : 3 * D], in_=ob[:, lo : 3 * D]
                        )
                # Pool probe: observe the last DVE copy's tick on the Pool
                # engine so the output DMA carries only the ACT copy wait.
                nc.gpsimd.tensor_copy(
                    pl_probe[0:1, 2 * s + 1 : 2 * s + 2], ob[0:1, 3 * D : 3 * D + 1]
                )
                lo = 3 * D if s != 0 else 0
                nc.gpsimd.dma_start(out=out[s, :, lo:], in_=ob[:, lo:])
                prev_ob = ob
    return nc


_PROGRAM_CACHE = {}


def _get_program(sched):
    key = tuple(tuple(tuple(g) for g in s) for s in sched)
    if key not in _PROGRAM_CACHE:
        _PROGRAM_CACHE[key] = _build_program(sched)
    return _PROGRAM_CACHE[key]


def _assign_slots(spans):
    """Assign the B samples to (slot, core) so that the per-slot UNION of
    (group, char-tile) matmul footprints is small: sort by profile, then
    cheap local-search swaps."""
    import random

    def union_cost(assign):
        total = 0
        for slot in assign:
            u = np.zeros((NG, NT), bool)
            for i in slot:
                for (g, t0, t1) in spans[i][0]:
                    u[g, t0 : t1 + 1] = True
            total += int(u.sum())
        return total

    order = sorted(range(B), key=lambda i: spans[i][1])
    best_assign, best = None, None
    for seed in range(6):
        rng = random.Random(seed)
        assign = [
            [order[s * N_CORES + c] for c in range(N_CORES)] for s in range(SPC)
        ]
        if seed:
            flat = [i for slot in assign for i in slot]
            rng.shuffle(flat)
            assign = [flat[s * N_CORES : (s + 1) * N_CORES] for s in range(SPC)]
        cost = union_cost(assign)
        for _ in range(30000):
            s1, s2 = rng.randrange(SPC), rng.randrange(SPC)
            if s1 == s2:
                continue
            i1, i2 = rng.randrange(N_CORES), rng.randrange(N_CORES)
            assign[s1][i1], assign[s2][i2] = assign[s2][i2], assign[s1][i1]
            c = union_cost(assign)
            if c <= cost:
                cost = c
            else:
                assign[s1][i1], assign[s2][i2] = assign[s2][i2], assign[s1][i1]
        if best is None or cost < best:
            best, best_assign = cost, [list(s) for s in assign]
    return best_assign


def _prep_inputs(feats, word_lens, seq_len, pos, pos_table):
    """Host-side metadata prep + batch sharding -> per-core input maps,
    union matmul schedule, and the sample->(slot, core) assignment."""
    feats = np.ascontiguousarray(np.asarray(feats), dtype=np.float32)
    word_lens = np.asarray(word_lens).astype(np.int64)
    seq_len = np.asarray(seq_len).astype(np.int64)
    pos = np.asarray(pos).astype(np.int64)
    pos_table = np.ascontiguousarray(np.asarray(pos_table), dtype=np.float32)

    wid = np.full((B, S), -1.0, np.float32)
    wrecw = np.zeros((B, W), np.float32)  # 1/len per word (0 for padding)
    lenw = np.zeros((B, W), np.float32)  # len per word (0 for padding)
    posoh = np.zeros((B, NPOS, W), np.float32)
    spans = []  # per sample: ([(g, t0, t1), ...], profile_key)
    for i in range(B):
        wl = word_lens[i]
        sl = int(seq_len[i])
        valid = wl != 0
        valid[0] = True
        ridx = np.nonzero(valid)[0]  # real words (contiguous prefix by construction)
        starts = wl[ridx]
        n = len(ridx)
        nxt = np.append(starts[1:], 0)
        ends = np.where(nxt == 0, sl, nxt)
        lens = np.maximum(ends - starts, 1)
        cidx = np.arange(sl)
        cwid = np.searchsorted(starts, cidx, side="right") - 1
        wid[i, :sl] = ridx[cwid].astype(np.float32)
        wrecw[i, ridx] = 1.0 / lens.astype(np.float32)
        lenw[i, ridx] = lens.astype(np.float32)
        posoh[i, pos[i], np.arange(W)] = 1.0  # one-hot part
        sp = []
        for g in range(NG):
            w0 = 128 * g
            if w0 >= n:
                continue
            w1 = min(128 * (g + 1), n)
            c0, c1 = int(starts[w0]), int(ends[w1 - 1])
            sp.append((g, c0 // 128, (c1 - 1) // 128))
        spans.append((sp, (n, sl)))

    assign = _assign_slots(spans)
    sched = []
    for s in range(SPC):
        u = np.zeros((NG, NT), bool)
        for i in assign[s]:
            for (g, t0, t1) in spans[i][0]:
                u[g, t0 : t1 + 1] = True
        sched.append(tuple(tuple(np.nonzero(u[g])[0].tolist()) for g in range(NG)))
    sched = tuple(sched)

    # [B, S] -> [B, 128, NT]: per-partition scalar columns per char tile
    widT = wid.reshape(B, NT, 128).transpose(0, 2, 1)
    # 1/len per word -> [B, 128, NG] per-partition scalars per word group
    wrecwT = wrecw.reshape(B, NG, 128).transpose(0, 2, 1)

    feats16 = feats.astype(np.float16)
    tab16 = pos_table.astype(np.float16)

    in_maps = []
    for c in range(N_CORES):
        cpb = np.zeros((64, CPB_W), np.float16)
        cpb[0:NPOS, CPB_TAB : CPB_TAB + D] = tab16
        cpr = np.zeros((128, CPR_W), np.float32)
        feats_c = np.empty((SPC, 128, NT * D), np.float16)
        for s in range(SPC):
            i = assign[s][c]
            # char c at partition c%128, cols (c//128)*D : (c//128+1)*D
            feats_c[s] = (
                feats16[i].reshape(NT, 128, D).transpose(1, 0, 2).reshape(128, NT * D)
            )
            cpr[:, CPR_META + NT * s : CPR_META + NT * (s + 1)] = widT[i]
            cpr[:, CPR_REC + NG * s : CPR_REC + NG * (s + 1)] = wrecwT[i]
            cpb[0:NPOS, CPB_POH + s * W : CPB_POH + (s + 1) * W] = (
                posoh[i] * lenw[i][None, :]
            ).astype(np.float16)
        in_maps.append({"feats": feats_c, "constpb": cpb, "constpr": cpr})
    return in_maps, sched, assign


def _run(in_maps, sched, assign, trace=False):
    from concourse.bass_utils import run_bass_kernel_spmd

    nc = _get_program(sched)
    res = run_bass_kernel_spmd(nc, in_maps, list(range(N_CORES)), trace=trace)
    out = np.empty((B, W, D), np.float32)
    for c in range(N_CORES):
        for s in range(SPC):
            # un-swizzle: word w at partition w%128, col block w//128
            o = res.results[c]["out"][s]  # [128, NG*D] fp16
            out[assign[s][c]] = (
                o.reshape(128, NG, D).transpose(1, 0, 2).reshape(W, D)
            ).astype(np.float32)
    return out, res


def kernel(feats, word_lens, seq_len, pos, pos_table):
    in_maps, sched, assign = _prep_inputs(feats, word_lens, seq_len, pos, pos_table)
    out, _ = _run(in_maps, sched, assign, trace=False)
    return out


# revision 14
# speedup vs baseline: 1.0699x; 1.0699x over previous
"""Trainium2 Bass kernel for CharModel ragged segment-mean + pos embedding.

Computation (per sample):
  out[j, :] = mean(feats[start_j:end_j, :]) * valid_j + pos_table[pos_j]
where the ragged segments are given by sorted word start offsets.

Strategy (fp16 data path; harness gate is rel_err < 2e-2 and fp16 lands
at ~7e-4, so no hi/lo compensation is needed):
  - Host precomputes per-char metadata: word_id[c] (which word each char
    belongs to, -1 for padding chars) and wrec[c] = 1/len(word(c)).
  - Device builds a one-hot matrix M[c, j] = (word_id[c]==j) with one
    fused DVE tensor_scalar op per 128-char tile, then the PE computes
    sum[W, D] = M.T @ feats directly in PSUM.  The pos embedding is
    added by accumulating (onehot(pos)*len).T @ pos_table into the same
    PSUM (both operands zero-padded to 128 contraction rows so the PE
    never switches tile quadrants), so the final 1/len multiply leaves
    mean + pos_table[pos].
  - feats ship pre-transposed in DRAM as [128, NT*D] fp16 (char c lives
    at partition c%128, tile c//128) so each per-sample feats DMA is
    per-partition contiguous: 128 descriptors instead of 128*NT, which
    cuts the HWDGE DIRECT2D issue cost ~4x.  The output is staged and
    written the same way ([128, NG*D] fp16, partition-contiguous) and
    un-swizzled on the host.
  - All DMAs ride the sync (HWDGE) queue with the metadata packs FIRST:
    SDMA rings drain in enqueue order, so const-after-feats would stall
    the warm-up + one-hot builds ~6us behind sample-0 feats data.
  - Data parallel over batch: 8 NeuronCores x 4 samples each, one shared
    SPMD program, per-core input maps.

Walrus ISA wait-slot limits dealt with throughout: matmul (S3_LW),
tensor_scalar (S3D3_TS) and DMA (PSEUDO_DMA_DIRECT2D) instructions can
carry only ONE semaphore wait each.  Hence:
  - the fp16 metadata (iota row + word-id scalars) ships in ONE packed
    tensor; the fp32 copy-scale reciprocals in a tiny second pack that a
    Scalar probe and a Vector probe observe once, so the PSUM->SBUF
    copies carry only their PE wait;
  - a per-sample 1x1 "gate" matmul (forced first in PE order via
    add_dep_helper, writing a dedicated never-overlapping PSUM region)
    alone carries the DVE wait for the lhsT builds and, transitively, the
    previous sample's PSUM bank releases;
  - feats tiles and output staging tiles get enough pool bufs that no
    slot is ever reused (no WAR waits on DMAs/copies);
  - two output DMAs per sample (SWDGE, each preceded by a Pool probe of
    the DVE-written half so the DMA carries only the ACT copy wait).
"""

import sys

if "/opt/trn_rl_repo" not in sys.path:
    sys.path.insert(0, "/opt/trn_rl_repo")

import numpy as np

import bass_rust
import concourse.bass as bass
import concourse.mybir as mybir
from concourse.tile import TileContext
from concourse.tile_sem_assignment import N_PROCS


class ChunkedDrainTileContext(TileContext):
    """TileContext whose kernel-tail drain is split into several drain
    instructions with one sem wait each (the CTRL_NO ISA struct rejects
    multi-wait drains here)."""

    DRAIN_CHUNK = 1

    def _drain_and_barrier(self, tick_clock, wait_clock):
        gc = tick_clock.global_clock
        ticks = [gc.peek_next(i) - 1 for i in range(N_PROCS)]
        active = [i for i, t in enumerate(ticks) if t > 0]
        for i in range(0, len(active), self.DRAIN_CHUNK):
            chunk = set(active[i : i + self.DRAIN_CHUNK])
            part = [ticks[j] if j in chunk else 0 for j in range(N_PROCS)]
            d = self.nc.sync.drain()
            wait_clock.add_sem_waits(
                d.ins, bass_rust.ScopedClock({None: bass_rust.VectorClock(part)})
            )
        self.nc.all_engine_barrier()
        assert self.sems is not None
        popped = self.nc._tile_sem_poison_stack.pop()
        assert popped is self._sem_poison
        self.nc.clear_and_free_semaphores(list(self.sems.allocated().values()))
        self.nc.all_engine_barrier()

B, S, D, W, NPOS = 32, 1024, 512 + 256, 512, 32  # D=768
N_CORES = 8
SPC = B // N_CORES  # samples per core
NT = S // 128  # char tiles per sample
NG = W // 128  # word groups per sample
CHUNKS = ((0, 384), (384, 384))  # D split for PSUM bank limit
F32 = mybir.dt.float32
F16 = mybir.dt.float16

# fp16 pos pack: only the 32 live rows ship over DMA; the zero rows
# 32:128 are memset on the DVE (shipping them would double the pack's
# critical-path DMA time)
CPB_POH = 0  # [32, SPC*W]: sample s cols s*W:(s+1)*W = onehot(pos)*len
CPB_TAB = SPC * W  # [32, D]: fp16(pos_table)
CPB_W = SPC * W + D

# fp32 pack (copy scales + word-id scalars; the is_equal build with f32
# stream + f32 scalar + fp16 out is the HW-proven config).  The iota row
# is generated on-device by the Pool engine, keeping this pack tiny so
# the one-hot builds start as early as possible.
CPR_REC = 0  # [128, NG*SPC]: per sample s: 1/len per word group
CPR_META = NG * SPC  # [128, NT*SPC]: per sample s: word-id per char
CPR_W = NG * SPC + NT * SPC


def _build_program(sched):
    """sched[s][g] = tuple of char-tile indices whose chars can touch word
    group g of slot-s samples on ANY core (union schedule; the one-hot
    lhsT zeroes contributions from tiles/words not actually present on a
    given core).  Matmuls for (g, t) pairs outside the schedule multiply
    all-zero one-hot slices and are skipped entirely."""
    nc = bass.Bass()
    # feats pre-transposed on host: char c of sample s lives at
    # [s, c % 128, (c // 128) * D : (c // 128 + 1) * D].
    feats = nc.declare_dram_parameter("feats", [SPC, 128, NT * D], F16, False)
    constpb = nc.declare_dram_parameter("constpb", [64, CPB_W], F16, False)
    constpr = nc.declare_dram_parameter("constpr", [128, CPR_W], F32, False)
    # output staged the same way: word w of sample s at
    # [s, w % 128, (w // 128) * D : ...]; host un-swizzles.
    out = nc.declare_dram_parameter("out", [SPC, 128, NG * D], F16, True)

    dep = lambda a, b, why: bass_rust.add_dep_helper(
        a.ins, b.ins, sync=False, reason=why
    )

    # per-sample used tile range (contiguous from 0 by construction)
    tmax = [max(t for g in range(NG) for t in sched[s][g]) for s in range(SPC)]
    with ChunkedDrainTileContext(nc) as tc:
        with (
            tc.tile_pool(name="const", bufs=1) as cpool,
            tc.tile_pool(name="feat", bufs=SPC) as fpool,
            tc.tile_pool(name="lhs", bufs=sum(t + 1 for t in tmax)) as lpool,
            tc.tile_pool(name="outsb", bufs=SPC) as opool,
            tc.tile_pool(name="psum", bufs=2 * NG - 1, space="PSUM") as ppool,
            tc.tile_pool(name="gatep", bufs=1, space="PSUM") as gpool,
        ):
            cpb = cpool.tile([128, CPB_W], F16)
            # iota row 0..W-1 on every partition, generated on-device
            # (fp32 is exact for these integers)
            iota_t = cpool.tile([128, W], F32)
            nc.gpsimd.iota(
                iota_t[:, :],
                pattern=[[1, W]],
                base=0,
                channel_multiplier=0,
                allow_small_or_imprecise_dtypes=True,
            )
            iota_f = iota_t[:, :]
            cpr = cpool.tile([128, CPR_W], F32)
            cpr_dma = nc.sync.dma_start(out=cpr[:, :], in_=constpr[:, :])
            # ACT probe: observe the f32-pack DMA on the Scalar engine once,
            # so the ACT PSUM->SBUF copies carry only their PE wait.  (The
            # DVE observes it via the first one-hot build.)  The DVE probe
            # observes the Pool iota tick so the builds carry only the
            # f32-pack wait.
            act_probe = cpool.tile([1, 2], F32)
            nc.scalar.copy(act_probe[0:1, 0:1], cpr[0:1, 0:1])
            nc.vector.tensor_copy(act_probe[0:1, 1:2], iota_t[0:1, 0:1])
            pl_probe = cpool.tile([1, 2 * SPC], F16)
            # cpb rows 64:128 zeroed on the Pool engine AFTER the iota (the
            # DMA ships rows 0:64, with 32:64 zero from the host); a tiny
            # "pgate" matmul below carries the memset tick for the pos
            # matmuls.
            nc.gpsimd.memset(cpb[64:128, :], 0.0)
            # Combined gate + warm-up PSUM bank: gates write cols 0:16,
            # the warm-up accumulates into cols 16:512 of row 0.
            gp = gpool.tile([128, 512], F32)
            gate_t = gp[:, 0:32]

            # PE warm-up: fat fp32 matmuls reading the on-device iota tile
            # run during the DMA ramp and trip the HAM clock gate before
            # the real matmuls start.  Without this the PE sometimes stays
            # at 1.2GHz for the whole kernel.
            for wi in range(5):
                nc.tensor.matmul(
                    gp[0:1, 32:224],
                    iota_t[:, 0:1],
                    iota_t[:, 0:192],
                    start=(wi == 0),
                    stop=(wi == 4),
                    skip_group_check=True,
                )
            # pgate: carries the Pool memset tick; every pos matmul deps on
            # it so none carries a second wait besides the cpb DMA sem.
            pgate = nc.tensor.matmul(
                gate_t[0:1, 29:30],
                cpb[64:65, 0:1],
                cpb[64:65, 0:1],
                start=True,
                stop=True,
                skip_group_check=True,
            )
            # feats DMAs: sample 0 in two pieces (so group-0/1 matmuls can
            # start as soon as their tiles land), later samples whole:
            # 7 sync-queue DMAs total, within the 8 HWDGE queues.
            fts = []
            for s in range(SPC):
                ft = fpool.tile([128, NT * D], F16, tag="ft", name=f"ft_{s}")
                fts.append(ft)
            # Input phasing: {cpr, s0} stream immediately; cpb starts once
            # the tiny f32 pack is in; {s1,s2,s3} start after the pos pack.
            # Each arrives just ahead of its consumer while the
            # head-critical packs get most of the bandwidth.
            ntile0 = tmax[0] + 1
            c1, c2 = (ntile0 + 2) // 3, (2 * ntile0 + 2) // 3
            nc.sync.dma_start(out=fts[0][:, : c1 * D], in_=feats[0, :, : c1 * D])
            cpb_dma = nc.sync.dma_start(out=cpb[0:64, :], in_=constpb[:, :])
            bass_rust.add_dep_helper(
                cpb_dma.ins, cpr_dma.ins, reason="pos pack after f32 pack"
            )
            nc.sync.dma_start(
                out=fts[0][:, c1 * D : c2 * D], in_=feats[0, :, c1 * D : c2 * D]
            )
            s0_dma = nc.sync.dma_start(
                out=fts[0][:, c2 * D : ntile0 * D],
                in_=feats[0, :, c2 * D : ntile0 * D],
            )
            for s in range(1, SPC):
                fd = nc.sync.dma_start(
                    out=fts[s][:, : (tmax[s] + 1) * D],
                    in_=feats[s, :, : (tmax[s] + 1) * D],
                )
                bass_rust.add_dep_helper(
                    fd.ins, s0_dma.ins, reason="feats tail after sample 0"
                )

            def emit_builds(s):
                d = {}
                for t in range(tmax[s] + 1):
                    lh = lpool.tile([128, W], F16, tag="lh", name=f"lh_{s}_{t}")
                    wcol = CPR_META + NT * s
                    nc.vector.tensor_scalar(
                        lh[:, :],
                        iota_f,
                        cpr[:, wcol + t : wcol + t + 1],
                        None,
                        op0=mybir.AluOpType.is_equal,
                    )
                    d[t] = lh
                return d

            lhs_by_s = {0: emit_builds(0)}
            prev_ob = None  # previous sample's output staging buffer
            for s in range(SPC):
                ft = fts[s]
                last_dve_copy = None
                last_act_copy = None
                ntile = tmax[s] + 1
                lhs = lhs_by_s[s]

                # Gate A: 1x1x1 matmul reading a late lhsT build; forced
                # first in PE order so it alone carries the DVE wait for
                # this sample's builds.  For sample 0 (head-critical) it
                # only covers the builds groups 0/1 consume; a second gate
                # at the halfway point covers the rest.
                if s == 0:
                    tA = max(sched[s][0])
                else:
                    tA = ntile - 1
                gate = nc.tensor.matmul(
                    gate_t[0:1, s : s + 1],
                    lhs[tA][0:1, 0:1],
                    lhs[tA][0:1, 0:1],
                    start=True,
                    stop=True,
                    skip_group_check=True,
                )
                # Early bank gates: cover the previous sample's PSUM bank
                # releases that this sample's FIRST half (units 0-3, prev
                # bufs 1-4) actually needs: the ACT copy chain through unit
                # 5 and the DVE copy chain through unit 6.  The last copies
                # of the previous sample are NOT waited on here, so the PE
                # does not stall at sample boundaries.
                if prev_ob is not None:
                    bgate = nc.tensor.matmul(
                        gate_t[0:1, SPC + s : SPC + s + 1],
                        prev_ob[0:1, 2 * D + 384 : 2 * D + 385],
                        prev_ob[0:1, 2 * D + 384 : 2 * D + 385],
                        start=True,
                        stop=True,
                        skip_group_check=True,
                    )
                    bgate2 = nc.tensor.matmul(
                        gate_t[0:1, 2 * SPC + s : 2 * SPC + s + 1],
                        prev_ob[0:1, 3 * D : 3 * D + 1],
                        prev_ob[0:1, 3 * D : 3 * D + 1],
                        start=True,
                        stop=True,
                        skip_group_check=True,
                    )
                else:
                    bgate = bgate2 = None

                ob = opool.tile([128, NG * D], F16, tag="ob", name=f"ob_{s}")
                gate2 = lgate1 = lgate2 = None
                gate_g1 = None
                for g in range(NG):
                    if g == 1 and s == 0:
                        # second build gate: covers the group-1 builds of
                        # sample 0 without delaying group 0 behind them
                        gate_g1 = nc.tensor.matmul(
                            gate_t[0:1, 30:31],
                            lhs[max(sched[s][1])][0:1, 0:1],
                            lhs[max(sched[s][1])][0:1, 0:1],
                            start=True,
                            stop=True,
                            skip_group_check=True,
                        )
                    if g == 2:
                        # Emit the NEXT sample's one-hot builds here: they
                        # slot into the DVE queue between this sample's
                        # unit-2 and unit-4 copies, so they never sit behind
                        # a copy that is blocked on a late PSUM (DVE
                        # head-of-line), and the next gateA finds them done.
                        if s + 1 < SPC:
                            lhs_by_s[s + 1] = emit_builds(s + 1)
                        # Late gates before the second half (units 4-7, which
                        # reuse the banks of prev units 5-7 and own unit 0):
                        # cover the previous sample's LAST ACT copy and this
                        # sample's first DVE copy.  For sample 0 a second
                        # build gate covers the groups-2/3 one-hot builds.
                        if s == 0:
                            gate2 = nc.tensor.matmul(
                                gate_t[0:1, 28:29],
                                lhs[ntile - 1][0:1, 0:1],
                                lhs[ntile - 1][0:1, 0:1],
                                start=True,
                                stop=True,
                                skip_group_check=True,
                            )
                        if prev_ob is not None:
                            lgate1 = nc.tensor.matmul(
                                gate_t[0:1, 3 * SPC + s : 3 * SPC + s + 1],
                                prev_ob[0:1, NG * D - 1 : NG * D],
                                prev_ob[0:1, NG * D - 1 : NG * D],
                                start=True,
                                stop=True,
                                skip_group_check=True,
                            )
                        lgate2 = nc.tensor.matmul(
                            gate_t[0:1, 4 * SPC + s : 4 * SPC + s + 1],
                            ob[0:1, 0:1],
                            ob[0:1, 0:1],
                            start=True,
                            stop=True,
                            skip_group_check=True,
                        )
                    tiles_g = sched[s][g]
                    pss = [
                        ppool.tile([128, cn], F32, tag="ps", name=f"ps_{s}_{g}_{ci}")
                        for ci, (c0, cn) in enumerate(CHUNKS)
                    ]

                    def _mm_deps(mm):
                        dep(mm, gate, "matmuls after sample gate")
                        if gate_g1 is not None and g == 1:
                            dep(mm, gate_g1, "matmuls after g1 build gate")
                        if gate2 is not None:
                            dep(mm, gate2, "matmuls after late build gate")
                        if g < 2:
                            if bgate is not None:
                                dep(mm, bgate, "matmuls after early bank gate")
                                dep(mm, bgate2, "matmuls after early bank gate2")
                        else:
                            if lgate1 is not None:
                                dep(mm, lgate1, "matmuls after late bank gate")
                            dep(mm, lgate2, "matmuls after late bank gate2")

                    # chunk-inner order: the two matmuls of a (g, t) pair use
                    # the same stationary weights back to back
                    for k, t in enumerate(tiles_g):
                        for ci, (c0, cn) in enumerate(CHUNKS):
                            mm = nc.tensor.matmul(
                                pss[ci][:, :],
                                lhs[t][:, 128 * g : 128 * (g + 1)],
                                ft[:, t * D + c0 : t * D + c0 + cn],
                                start=(k == 0),
                                stop=False,
                                skip_group_check=True,
                            )
                            _mm_deps(mm)
                    # pos contribution scaled by len (exact in fp16) so the
                    # final 1/len multiply leaves pos_table[pos] exactly;
                    # both operands are zero-padded to 128 rows
                    pcol = CPB_POH + s * W
                    for ci, (c0, cn) in enumerate(CHUNKS):
                        mm = nc.tensor.matmul(
                            pss[ci][:, :],
                            cpb[:, pcol + 128 * g : pcol + 128 * (g + 1)],
                            cpb[:, CPB_TAB + c0 : CPB_TAB + c0 + cn],
                            start=(len(tiles_g) == 0),
                            stop=True,
                            skip_group_check=True,
                        )
                        dep(mm, pgate, "pos matmul after memset gate")
                        _mm_deps(mm)
                    for ci, (c0, cn) in enumerate(CHUNKS):
                        unit = 2 * g + ci
                        recip_ap = cpr[
                            :, CPR_REC + NG * s + g : CPR_REC + NG * s + g + 1
                        ]
                        # last sample: units 0-3 all on ACT so the g1-point
                        # output DMA needs only the chained unit-3 ACT wait
                        use_dve = (unit % 2 == 0) and not (s == SPC - 1 and unit < 4)
                        if use_dve:
                            cp = nc.vector.tensor_scalar(
                                ob[:, g * D + c0 : g * D + c0 + cn],
                                pss[ci][:, :],
                                recip_ap,
                                None,
                                op0=mybir.AluOpType.mult,
                            )
                            if last_dve_copy is not None:
                                dep(cp, last_dve_copy, "DVE copy order")
                            last_dve_copy = cp
                        else:
                            cp = nc.scalar.activation(
                                ob[:, g * D + c0 : g * D + c0 + cn],
                                pss[ci][:, :],
                                mybir.ActivationFunctionType.Copy,
                                scale=recip_ap,
                            )
                            if last_act_copy is not None:
                                dep(cp, last_act_copy, "ACT copy order")
                            last_act_copy = cp
                    # streaming output: sample 0 ships once at the end
                    # (it is never tail-critical); middle samples split at
                    # the g2 point; the last sample ships three pieces so
                    # most of its output flows during its own compute.
                    # Pool probes observe the DVE copy chain so each DMA
                    # carries only the chained ACT wait.
                    last = s == SPC - 1
                    if last and g == 1:
                        # units 0-3 are all ACT copies: the DMA's unit-3
                        # wait covers them by the ACT dep chain
                        nc.gpsimd.dma_start(
                            out=out[s, :, : 2 * D], in_=ob[:, : 2 * D]
                        )
                    if g == 2 and s != 0:
                        lo = 2 * D if last else 0
                        nc.gpsimd.tensor_copy(
                            pl_probe[0:1, 2 * s : 2 * s + 1],
                            ob[0:1, 2 * D : 2 * D + 1],
                        )
                        nc.gpsimd.dma_start(
                            out=out[s, :, lo : 3 * D], in_=ob[:, lo : 3 * D]
                        )
                # Pool probe: observe the last DVE copy's tick on the Pool
                # engine so the output DMA carries only the ACT copy wait.
                nc.gpsimd.tensor_copy(
                    pl_probe[0:1, 2 * s + 1 : 2 * s + 2], ob[0:1, 3 * D : 3 * D + 1]
                )
                lo = 3 * D if s != 0 else 0
                nc.gpsimd.dma_start(out=out[s, :, lo:], in_=ob[:, lo:])
                prev_ob = ob
    return nc


_PROGRAM_CACHE = {}


def _get_program(sched):
    key = tuple(tuple(tuple(g) for g in s) for s in sched)
    if key not in _PROGRAM_CACHE:
        _PROGRAM_CACHE[key] = _build_program(sched)
    return _PROGRAM_CACHE[key]


def _assign_slots(spans):
    """Assign the B samples to (slot, core) so that the per-slot UNION of
    (group, char-tile) matmul footprints is small: sort by profile, then
    cheap local-search swaps."""
    import random

    def union_cost(assign):
        total = 0
        for slot in assign:
            u = np.zeros((NG, NT), bool)
            for i in slot:
                for (g, t0, t1) in spans[i][0]:
                    u[g, t0 : t1 + 1] = True
            total += int(u.sum())
        return total

    order = sorted(range(B), key=lambda i: spans[i][1])
    best_assign, best = None, None
    for seed in range(6):
        rng = random.Random(seed)
        assign = [
            [order[s * N_CORES + c] for c in range(N_CORES)] for s in range(SPC)
        ]
        if seed:
            flat = [i for slot in assign for i in slot]
            rng.shuffle(flat)
            assign = [flat[s * N_CORES : (s + 1) * N_CORES] for s in range(SPC)]
        cost = union_cost(assign)
        for _ in range(30000):
            s1, s2 = rng.randrange(SPC), rng.randrange(SPC)
            if s1 == s2:
                continue
            i1, i2 = rng.randrange(N_CORES), rng.randrange(N_CORES)
            assign[s1][i1], assign[s2][i2] = assign[s2][i2], assign[s1][i1]
            c = union_cost(assign)
            if c <= cost:
                cost = c
            else:
                assign[s1][i1], assign[s2][i2] = assign[s2][i2], assign[s1][i1]
        if best is None or cost < best:
            best, best_assign = cost, [list(s) for s in assign]
    return best_assign


def _prep_inputs(feats, word_lens, seq_len, pos, pos_table):
    """Host-side metadata prep + batch sharding -> per-core input maps,
    union matmul schedule, and the sample->(slot, core) assignment."""
    feats = np.ascontiguousarray(np.asarray(feats), dtype=np.float32)
    word_lens = np.asarray(word_lens).astype(np.int64)
    seq_len = np.asarray(seq_len).astype(np.int64)
    pos = np.asarray(pos).astype(np.int64)
    pos_table = np.ascontiguousarray(np.asarray(pos_table), dtype=np.float32)

    wid = np.full((B, S), -1.0, np.float32)
    wrecw = np.zeros((B, W), np.float32)  # 1/len per word (0 for padding)
    lenw = np.zeros((B, W), np.float32)  # len per word (0 for padding)
    posoh = np.zeros((B, NPOS, W), np.float32)
    spans = []  # per sample: ([(g, t0, t1), ...], profile_key)
    for i in range(B):
        wl = word_lens[i]
        sl = int(seq_len[i])
        valid = wl != 0
        valid[0] = True
        ridx = np.nonzero(valid)[0]  # real words (contiguous prefix by construction)
        starts = wl[ridx]
        n = len(ridx)
        nxt = np.append(starts[1:], 0)
        ends = np.where(nxt == 0, sl, nxt)
        lens = np.maximum(ends - starts, 1)
        cidx = np.arange(sl)
        cwid = np.searchsorted(starts, cidx, side="right") - 1
        wid[i, :sl] = ridx[cwid].astype(np.float32)
        wrecw[i, ridx] = 1.0 / lens.astype(np.float32)
        lenw[i, ridx] = lens.astype(np.float32)
        posoh[i, pos[i], np.arange(W)] = 1.0  # one-hot part
        sp = []
        for g in range(NG):
            w0 = 128 * g
            if w0 >= n:
                continue
            w1 = min(128 * (g + 1), n)
            c0, c1 = int(starts[w0]), int(ends[w1 - 1])
            sp.append((g, c0 // 128, (c1 - 1) // 128))
        spans.append((sp, (n, sl)))

    assign = _assign_slots(spans)
    sched = []
    for s in range(SPC):
        u = np.zeros((NG, NT), bool)
        for i in assign[s]:
            for (g, t0, t1) in spans[i][0]:
                u[g, t0 : t1 + 1] = True
        sched.append(tuple(tuple(np.nonzero(u[g])[0].tolist()) for g in range(NG)))
    sched = tuple(sched)

    # [B, S] -> [B, 128, NT]: per-partition scalar columns per char tile
    widT = wid.reshape(B, NT, 128).transpose(0, 2, 1)
    # 1/len per word -> [B, 128, NG] per-partition scalars per word group
    wrecwT = wrecw.reshape(B, NG, 128).transpose(0, 2, 1)

    feats16 = feats.astype(np.float16)
    tab16 = pos_table.astype(np.float16)

    in_maps = []
    for c in range(N_CORES):
        cpb = np.zeros((64, CPB_W), np.float16)
        cpb[0:NPOS, CPB_TAB : CPB_TAB + D] = tab16
        cpr = np.zeros((128, CPR_W), np.float32)
        feats_c = np.empty((SPC, 128, NT * D), np.float16)
        for s in range(SPC):
            i = assign[s][c]
            # char c at partition c%128, cols (c//128)*D : (c//128+1)*D
            feats_c[s] = (
                feats16[i].reshape(NT, 128, D).transpose(1, 0, 2).reshape(128, NT * D)
            )
            cpr[:, CPR_META + NT * s : CPR_META + NT * (s + 1)] = widT[i]
            cpr[:, CPR_REC + NG * s : CPR_REC + NG * (s + 1)] = wrecwT[i]
            cpb[0:NPOS, CPB_POH + s * W : CPB_POH + (s + 1) * W] = (
                posoh[i] * lenw[i][None, :]
            ).astype(np.float16)
        in_maps.append({"feats": feats_c, "constpb": cpb, "constpr": cpr})
    return in_maps, sched, assign


def _run(in_maps, sched, assign, trace=False):
    from concourse.bass_utils import run_bass_kernel_spmd

    nc = _get_program(sched)
    res = run_bass_kernel_spmd(nc, in_maps, list(range(N_CORES)), trace=trace)
    out = np.empty((B, W, D), np.float32)
    for c in range(N_CORES):
        for s in range(SPC):
            # un-swizzle: word w at partition w%128, col block w//128
            o = res.results[c]["out"][s]  # [128, NG*D] fp16
            out[assign[s][c]] = (
                o.reshape(128, NG, D).transpose(1, 0, 2).reshape(W, D)
            ).astype(np.float32)
    return out, res


def kernel(feats, word_lens, seq_len, pos, pos_table):
    in_maps, sched, assign = _prep_inputs(feats, word_lens, seq_len, pos, pos_table)
    out, _ = _run(in_maps, sched, assign, trace=False)
    return out


# revision 15
# speedup vs baseline: 1.0716x; 1.0016x over previous
"""Trainium2 Bass kernel for CharModel ragged segment-mean + pos embedding.

Computation (per sample):
  out[j, :] = mean(feats[start_j:end_j, :]) * valid_j + pos_table[pos_j]
where the ragged segments are given by sorted word start offsets.

Strategy (fp16 data path; harness gate is rel_err < 2e-2 and fp16 lands
at ~7e-4, so no hi/lo compensation is needed):
  - Host precomputes per-char metadata: word_id[c] (which word each char
    belongs to, -1 for padding chars) and wrec[c] = 1/len(word(c)).
  - Device builds a one-hot matrix M[c, j] = (word_id[c]==j) with one
    fused DVE tensor_scalar op per 128-char tile, then the PE computes
    sum[W, D] = M.T @ feats directly in PSUM.  The pos embedding is
    added by accumulating (onehot(pos)*len).T @ pos_table into the same
    PSUM (both operands zero-padded to 128 contraction rows so the PE
    never switches tile quadrants), so the final 1/len multiply leaves
    mean + pos_table[pos].
  - feats ship pre-transposed in DRAM as [128, NT*D] fp16 (char c lives
    at partition c%128, tile c//128) so each per-sample feats DMA is
    per-partition contiguous: 128 descriptors instead of 128*NT, which
    cuts the HWDGE DIRECT2D issue cost ~4x.  The output is staged and
    written the same way ([128, NG*D] fp16, partition-contiguous) and
    un-swizzled on the host.
  - All DMAs ride the sync (HWDGE) queue with the metadata packs FIRST:
    SDMA rings drain in enqueue order, so const-after-feats would stall
    the warm-up + one-hot builds ~6us behind sample-0 feats data.
  - Data parallel over batch: 8 NeuronCores x 4 samples each, one shared
    SPMD program, per-core input maps.

Walrus ISA wait-slot limits dealt with throughout: matmul (S3_LW),
tensor_scalar (S3D3_TS) and DMA (PSEUDO_DMA_DIRECT2D) instructions can
carry only ONE semaphore wait each.  Hence:
  - the fp16 metadata (iota row + word-id scalars) ships in ONE packed
    tensor; the fp32 copy-scale reciprocals in a tiny second pack that a
    Scalar probe and a Vector probe observe once, so the PSUM->SBUF
    copies carry only their PE wait;
  - a per-sample 1x1 "gate" matmul (forced first in PE order via
    add_dep_helper, writing a dedicated never-overlapping PSUM region)
    alone carries the DVE wait for the lhsT builds and, transitively, the
    previous sample's PSUM bank releases;
  - feats tiles and output staging tiles get enough pool bufs that no
    slot is ever reused (no WAR waits on DMAs/copies);
  - two output DMAs per sample (SWDGE, each preceded by a Pool probe of
    the DVE-written half so the DMA carries only the ACT copy wait).
"""

import sys

if "/opt/trn_rl_repo" not in sys.path:
    sys.path.insert(0, "/opt/trn_rl_repo")

import numpy as np

import bass_rust
import concourse.bass as bass
import concourse.mybir as mybir
from concourse.tile import TileContext
from concourse.tile_sem_assignment import N_PROCS


class ChunkedDrainTileContext(TileContext):
    """TileContext whose kernel-tail drain is split into several drain
    instructions with one sem wait each (the CTRL_NO ISA struct rejects
    multi-wait drains here)."""

    DRAIN_CHUNK = 1

    def _drain_and_barrier(self, tick_clock, wait_clock):
        gc = tick_clock.global_clock
        ticks = [gc.peek_next(i) - 1 for i in range(N_PROCS)]
        active = [i for i, t in enumerate(ticks) if t > 0]
        for i in range(0, len(active), self.DRAIN_CHUNK):
            chunk = set(active[i : i + self.DRAIN_CHUNK])
            part = [ticks[j] if j in chunk else 0 for j in range(N_PROCS)]
            d = self.nc.sync.drain()
            wait_clock.add_sem_waits(
                d.ins, bass_rust.ScopedClock({None: bass_rust.VectorClock(part)})
            )
        self.nc.all_engine_barrier()
        assert self.sems is not None
        popped = self.nc._tile_sem_poison_stack.pop()
        assert popped is self._sem_poison
        self.nc.clear_and_free_semaphores(list(self.sems.allocated().values()))
        self.nc.all_engine_barrier()

B, S, D, W, NPOS = 32, 1024, 512 + 256, 512, 32  # D=768
N_CORES = 8
SPC = B // N_CORES  # samples per core
NT = S // 128  # char tiles per sample
NG = W // 128  # word groups per sample
CHUNKS = ((0, 384), (384, 384))  # D split for PSUM bank limit
F32 = mybir.dt.float32
F16 = mybir.dt.float16

# fp16 pos pack: only the 32 live rows ship over DMA; the zero rows
# 32:128 are memset on the DVE (shipping them would double the pack's
# critical-path DMA time)
CPB_POH = 0  # [32, SPC*W]: sample s cols s*W:(s+1)*W = onehot(pos)*len
CPB_TAB = SPC * W  # [32, D]: fp16(pos_table)
CPB_W = SPC * W + D

# fp32 pack (copy scales + word-id scalars; the is_equal build with f32
# stream + f32 scalar + fp16 out is the HW-proven config).  The iota row
# is generated on-device by the Pool engine, keeping this pack tiny so
# the one-hot builds start as early as possible.
CPR_REC = 0  # [128, NG*SPC]: per sample s: 1/len per word group
CPR_META = NG * SPC  # [128, NT*SPC]: per sample s: word-id per char
CPR_W = NG * SPC + NT * SPC


def _build_program(sched):
    """sched[s][g] = tuple of char-tile indices whose chars can touch word
    group g of slot-s samples on ANY core (union schedule; the one-hot
    lhsT zeroes contributions from tiles/words not actually present on a
    given core).  Matmuls for (g, t) pairs outside the schedule multiply
    all-zero one-hot slices and are skipped entirely."""
    nc = bass.Bass()
    # feats pre-transposed on host: char c of sample s lives at
    # [s, c % 128, (c // 128) * D : (c // 128 + 1) * D].
    feats = nc.declare_dram_parameter("feats", [SPC, 128, NT * D], F16, False)
    constpb = nc.declare_dram_parameter("constpb", [64, CPB_W], F16, False)
    constpr = nc.declare_dram_parameter("constpr", [128, CPR_W], F32, False)
    # output staged the same way: word w of sample s at
    # [s, w % 128, (w // 128) * D : ...]; host un-swizzles.
    out = nc.declare_dram_parameter("out", [SPC, 128, NG * D], F16, True)

    dep = lambda a, b, why: bass_rust.add_dep_helper(
        a.ins, b.ins, sync=False, reason=why
    )

    # per-sample used tile range (contiguous from 0 by construction)
    tmax = [max(t for g in range(NG) for t in sched[s][g]) for s in range(SPC)]
    with ChunkedDrainTileContext(nc) as tc:
        with (
            tc.tile_pool(name="const", bufs=1) as cpool,
            tc.tile_pool(name="feat", bufs=SPC) as fpool,
            tc.tile_pool(name="lhs", bufs=sum(t + 1 for t in tmax)) as lpool,
            tc.tile_pool(name="outsb", bufs=SPC) as opool,
            tc.tile_pool(name="psum", bufs=2 * NG - 1, space="PSUM") as ppool,
            tc.tile_pool(name="gatep", bufs=1, space="PSUM") as gpool,
        ):
            cpb = cpool.tile([128, CPB_W], F16)
            # iota row 0..W-1 on every partition, generated on-device
            # (fp32 is exact for these integers)
            iota_t = cpool.tile([128, W], F32)
            nc.gpsimd.iota(
                iota_t[:, :],
                pattern=[[1, W]],
                base=0,
                channel_multiplier=0,
                allow_small_or_imprecise_dtypes=True,
            )
            iota_f = iota_t[:, :]
            cpr = cpool.tile([128, CPR_W], F32)
            cpr_dma = nc.sync.dma_start(out=cpr[:, :], in_=constpr[:, :])
            # ACT probe: observe the f32-pack DMA on the Scalar engine once,
            # so the ACT PSUM->SBUF copies carry only their PE wait.  (The
            # DVE observes it via the first one-hot build.)  The DVE probe
            # observes the Pool iota tick so the builds carry only the
            # f32-pack wait.
            act_probe = cpool.tile([1, 2], F32)
            nc.scalar.copy(act_probe[0:1, 0:1], cpr[0:1, 0:1])
            nc.vector.tensor_copy(act_probe[0:1, 1:2], iota_t[0:1, 0:1])
            pl_probe = cpool.tile([1, 2 * SPC], F16)
            # cpb rows 64:128 zeroed on the Pool engine AFTER the iota (the
            # DMA ships rows 0:64, with 32:64 zero from the host); a tiny
            # "pgate" matmul below carries the memset tick for the pos
            # matmuls.
            nc.gpsimd.memset(cpb[64:128, :], 0.0)
            # Combined gate + warm-up PSUM bank: gates write cols 0:16,
            # the warm-up accumulates into cols 16:512 of row 0.
            gp = gpool.tile([128, 512], F32)
            gate_t = gp[:, 0:32]

            # PE warm-up: fat fp32 matmuls reading the on-device iota tile
            # run during the DMA ramp and trip the HAM clock gate before
            # the real matmuls start.  Without this the PE sometimes stays
            # at 1.2GHz for the whole kernel.
            for wi in range(5):
                nc.tensor.matmul(
                    gp[0:1, 32:224],
                    iota_t[:, 0:1],
                    iota_t[:, 0:192],
                    start=(wi == 0),
                    stop=(wi == 4),
                    skip_group_check=True,
                )
            # pgate: carries the Pool memset tick; every pos matmul deps on
            # it so none carries a second wait besides the cpb DMA sem.
            pgate = nc.tensor.matmul(
                gate_t[0:1, 29:30],
                cpb[64:65, 0:1],
                cpb[64:65, 0:1],
                start=True,
                stop=True,
                skip_group_check=True,
            )
            # feats DMAs: sample 0 in two pieces (so group-0/1 matmuls can
            # start as soon as their tiles land), later samples whole:
            # 7 sync-queue DMAs total, within the 8 HWDGE queues.
            fts = []
            for s in range(SPC):
                ft = fpool.tile([128, NT * D], F16, tag="ft", name=f"ft_{s}")
                fts.append(ft)
            # Input phasing: {cpr, s0} stream immediately; cpb starts once
            # the tiny f32 pack is in; {s1,s2,s3} start after the pos pack.
            # Each arrives just ahead of its consumer while the
            # head-critical packs get most of the bandwidth.
            ntile0 = tmax[0] + 1
            c1, c2 = min(3, ntile0 - 1), ntile0 - 1
            nc.sync.dma_start(out=fts[0][:, : c1 * D], in_=feats[0, :, : c1 * D])
            cpb_dma = nc.sync.dma_start(out=cpb[0:64, :], in_=constpb[:, :])
            bass_rust.add_dep_helper(
                cpb_dma.ins, cpr_dma.ins, reason="pos pack after f32 pack"
            )
            nc.sync.dma_start(
                out=fts[0][:, c1 * D : c2 * D], in_=feats[0, :, c1 * D : c2 * D]
            )
            s0_dma = nc.sync.dma_start(
                out=fts[0][:, c2 * D : ntile0 * D],
                in_=feats[0, :, c2 * D : ntile0 * D],
            )
            for s in range(1, SPC):
                fd = nc.sync.dma_start(
                    out=fts[s][:, : (tmax[s] + 1) * D],
                    in_=feats[s, :, : (tmax[s] + 1) * D],
                )
                bass_rust.add_dep_helper(
                    fd.ins, s0_dma.ins, reason="feats tail after sample 0"
                )

            def emit_builds(s):
                d = {}
                for t in range(tmax[s] + 1):
                    lh = lpool.tile([128, W], F16, tag="lh", name=f"lh_{s}_{t}")
                    wcol = CPR_META + NT * s
                    nc.vector.tensor_scalar(
                        lh[:, :],
                        iota_f,
                        cpr[:, wcol + t : wcol + t + 1],
                        None,
                        op0=mybir.AluOpType.is_equal,
                    )
                    d[t] = lh
                return d

            lhs_by_s = {0: emit_builds(0)}
            prev_ob = None  # previous sample's output staging buffer
            for s in range(SPC):
                ft = fts[s]
                last_dve_copy = None
                last_act_copy = None
                ntile = tmax[s] + 1
                lhs = lhs_by_s[s]

                # Gate A: 1x1x1 matmul reading a late lhsT build; forced
                # first in PE order so it alone carries the DVE wait for
                # this sample's builds.  For sample 0 (head-critical) it
                # only covers the builds groups 0/1 consume; a second gate
                # at the halfway point covers the rest.
                if s == 0:
                    tA = max(sched[s][0])
                else:
                    tA = ntile - 1
                gate = nc.tensor.matmul(
                    gate_t[0:1, s : s + 1],
                    lhs[tA][0:1, 0:1],
                    lhs[tA][0:1, 0:1],
                    start=True,
                    stop=True,
                    skip_group_check=True,
                )
                # Early bank gates: cover the previous sample's PSUM bank
                # releases that this sample's FIRST half (units 0-3, prev
                # bufs 1-4) actually needs: the ACT copy chain through unit
                # 5 and the DVE copy chain through unit 6.  The last copies
                # of the previous sample are NOT waited on here, so the PE
                # does not stall at sample boundaries.
                if prev_ob is not None:
                    bgate = nc.tensor.matmul(
                        gate_t[0:1, SPC + s : SPC + s + 1],
                        prev_ob[0:1, 2 * D + 384 : 2 * D + 385],
                        prev_ob[0:1, 2 * D + 384 : 2 * D + 385],
                        start=True,
                        stop=True,
                        skip_group_check=True,
                    )
                    bgate2 = nc.tensor.matmul(
                        gate_t[0:1, 2 * SPC + s : 2 * SPC + s + 1],
                        prev_ob[0:1, 3 * D : 3 * D + 1],
                        prev_ob[0:1, 3 * D : 3 * D + 1],
                        start=True,
                        stop=True,
                        skip_group_check=True,
                    )
                else:
                    bgate = bgate2 = None

                ob = opool.tile([128, NG * D], F16, tag="ob", name=f"ob_{s}")
                gate2 = lgate1 = lgate2 = None
                gate_g1 = None
                for g in range(NG):
                    if g == 1 and s == 0:
                        # second build gate: covers the group-1 builds of
                        # sample 0 without delaying group 0 behind them
                        gate_g1 = nc.tensor.matmul(
                            gate_t[0:1, 30:31],
                            lhs[max(sched[s][1])][0:1, 0:1],
                            lhs[max(sched[s][1])][0:1, 0:1],
                            start=True,
                            stop=True,
                            skip_group_check=True,
                        )
                    if g == 2:
                        # Emit the NEXT sample's one-hot builds here: they
                        # slot into the DVE queue between this sample's
                        # unit-2 and unit-4 copies, so they never sit behind
                        # a copy that is blocked on a late PSUM (DVE
                        # head-of-line), and the next gateA finds them done.
                        if s + 1 < SPC:
                            lhs_by_s[s + 1] = emit_builds(s + 1)
                        # For sample 0 a second build gate covers the
                        # groups-2/3 one-hot builds.
                        if s == 0:
                            gate2 = nc.tensor.matmul(
                                gate_t[0:1, 28:29],
                                lhs[ntile - 1][0:1, 0:1],
                                lhs[ntile - 1][0:1, 0:1],
                                start=True,
                                stop=True,
                                skip_group_check=True,
                            )
                    if g == 3:
                        # Late bank gates, needed only by group 3: unit 6
                        # reuses the bank of the previous sample's unit 7
                        # and unit 7 reuses this sample's unit-0 bank.
                        if prev_ob is not None:
                            lgate1 = nc.tensor.matmul(
                                gate_t[0:1, 3 * SPC + s : 3 * SPC + s + 1],
                                prev_ob[0:1, NG * D - 1 : NG * D],
                                prev_ob[0:1, NG * D - 1 : NG * D],
                                start=True,
                                stop=True,
                                skip_group_check=True,
                            )
                        lgate2 = nc.tensor.matmul(
                            gate_t[0:1, 4 * SPC + s : 4 * SPC + s + 1],
                            ob[0:1, 0:1],
                            ob[0:1, 0:1],
                            start=True,
                            stop=True,
                            skip_group_check=True,
                        )
                    tiles_g = sched[s][g]
                    pss = [
                        ppool.tile([128, cn], F32, tag="ps", name=f"ps_{s}_{g}_{ci}")
                        for ci, (c0, cn) in enumerate(CHUNKS)
                    ]

                    def _mm_deps(mm):
                        dep(mm, gate, "matmuls after sample gate")
                        if gate_g1 is not None and g == 1:
                            dep(mm, gate_g1, "matmuls after g1 build gate")
                        if gate2 is not None:
                            dep(mm, gate2, "matmuls after late build gate")
                        if bgate is not None and g < 3:
                            dep(mm, bgate, "matmuls after early bank gate")
                            dep(mm, bgate2, "matmuls after early bank gate2")
                        if g == 3:
                            if lgate1 is not None:
                                dep(mm, lgate1, "matmuls after late bank gate")
                            dep(mm, lgate2, "matmuls after late bank gate2")

                    # chunk-inner order: the two matmuls of a (g, t) pair use
                    # the same stationary weights back to back
                    for k, t in enumerate(tiles_g):
                        for ci, (c0, cn) in enumerate(CHUNKS):
                            mm = nc.tensor.matmul(
                                pss[ci][:, :],
                                lhs[t][:, 128 * g : 128 * (g + 1)],
                                ft[:, t * D + c0 : t * D + c0 + cn],
                                start=(k == 0),
                                stop=False,
                                skip_group_check=True,
                            )
                            _mm_deps(mm)
                    # pos contribution scaled by len (exact in fp16) so the
                    # final 1/len multiply leaves pos_table[pos] exactly;
                    # both operands are zero-padded to 128 rows
                    pcol = CPB_POH + s * W
                    for ci, (c0, cn) in enumerate(CHUNKS):
                        mm = nc.tensor.matmul(
                            pss[ci][:, :],
                            cpb[:, pcol + 128 * g : pcol + 128 * (g + 1)],
                            cpb[:, CPB_TAB + c0 : CPB_TAB + c0 + cn],
                            start=(len(tiles_g) == 0),
                            stop=True,
                            skip_group_check=True,
                        )
                        dep(mm, pgate, "pos matmul after memset gate")
                        _mm_deps(mm)
                    for ci, (c0, cn) in enumerate(CHUNKS):
                        unit = 2 * g + ci
                        recip_ap = cpr[
                            :, CPR_REC + NG * s + g : CPR_REC + NG * s + g + 1
                        ]
                        # last sample: units 0-3 all on ACT so the g1-point
                        # output DMA needs only the chained unit-3 ACT wait
                        use_dve = (unit % 2 == 0) and not (s == SPC - 1 and unit < 4)
                        if use_dve:
                            cp = nc.vector.tensor_scalar(
                                ob[:, g * D + c0 : g * D + c0 + cn],
                                pss[ci][:, :],
                                recip_ap,
                                None,
                                op0=mybir.AluOpType.mult,
                            )
                            if last_dve_copy is not None:
                                dep(cp, last_dve_copy, "DVE copy order")
                            last_dve_copy = cp
                        else:
                            cp = nc.scalar.activation(
                                ob[:, g * D + c0 : g * D + c0 + cn],
                                pss[ci][:, :],
                                mybir.ActivationFunctionType.Copy,
                                scale=recip_ap,
                            )
                            if last_act_copy is not None:
                                dep(cp, last_act_copy, "ACT copy order")
                            last_act_copy = cp
                    # streaming output: sample 0 ships once at the end
                    # (it is never tail-critical); middle samples split at
                    # the g2 point; the last sample ships three pieces so
                    # most of its output flows during its own compute.
                    # Pool probes observe the DVE copy chain so each DMA
                    # carries only the chained ACT wait.
                    last = s == SPC - 1
                    if last and g == 1:
                        # units 0-3 are all ACT copies: the DMA's unit-3
                        # wait covers them by the ACT dep chain
                        nc.gpsimd.dma_start(
                            out=out[s, :, : 2 * D], in_=ob[:, : 2 * D]
                        )
                    if g == 2 and s != 0:
                        lo = 2 * D if last else 0
                        nc.gpsimd.tensor_copy(
                            pl_probe[0:1, 2 * s : 2 * s + 1],
                            ob[0:1, 2 * D : 2 * D + 1],
                        )
                        nc.gpsimd.dma_start(
                            out=out[s, :, lo : 3 * D], in_=ob[:, lo : 3 * D]
                        )
                # Pool probe: observe the last DVE copy's tick on the Pool
                # engine so the output DMA carries only the ACT copy wait.
                nc.gpsimd.tensor_copy(
                    pl_probe[0:1, 2 * s + 1 : 2 * s + 2], ob[0:1, 3 * D : 3 * D + 1]
                )
                lo = 3 * D if s != 0 else 0
                nc.gpsimd.dma_start(out=out[s, :, lo:], in_=ob[:, lo:])
                prev_ob = ob
    return nc


_PROGRAM_CACHE = {}


def _get_program(sched):
    key = tuple(tuple(tuple(g) for g in s) for s in sched)
    if key not in _PROGRAM_CACHE:
        _PROGRAM_CACHE[key] = _build_program(sched)
    return _PROGRAM_CACHE[key]


def _assign_slots(spans):
    """Assign the B samples to (slot, core) so that the per-slot UNION of
    (group, char-tile) matmul footprints is small: sort by profile, then
    cheap local-search swaps."""
    import random

    def union_cost(assign):
        total = 0
        for slot in assign:
            u = np.zeros((NG, NT), bool)
            for i in slot:
                for (g, t0, t1) in spans[i][0]:
                    u[g, t0 : t1 + 1] = True
            total += int(u.sum())
        return total

    order = sorted(range(B), key=lambda i: spans[i][1])
    best_assign, best = None, None
    for seed in range(6):
        rng = random.Random(seed)
        assign = [
            [order[s * N_CORES + c] for c in range(N_CORES)] for s in range(SPC)
        ]
        if seed:
            flat = [i for slot in assign for i in slot]
            rng.shuffle(flat)
            assign = [flat[s * N_CORES : (s + 1) * N_CORES] for s in range(SPC)]
        cost = union_cost(assign)
        for _ in range(30000):
            s1, s2 = rng.randrange(SPC), rng.randrange(SPC)
            if s1 == s2:
                continue
            i1, i2 = rng.randrange(N_CORES), rng.randrange(N_CORES)
            assign[s1][i1], assign[s2][i2] = assign[s2][i2], assign[s1][i1]
            c = union_cost(assign)
            if c <= cost:
                cost = c
            else:
                assign[s1][i1], assign[s2][i2] = assign[s2][i2], assign[s1][i1]
        if best is None or cost < best:
            best, best_assign = cost, [list(s) for s in assign]
    return best_assign


def _prep_inputs(feats, word_lens, seq_len, pos, pos_table):
    """Host-side metadata prep + batch sharding -> per-core input maps,
    union matmul schedule, and the sample->(slot, core) assignment."""
    feats = np.ascontiguousarray(np.asarray(feats), dtype=np.float32)
    word_lens = np.asarray(word_lens).astype(np.int64)
    seq_len = np.asarray(seq_len).astype(np.int64)
    pos = np.asarray(pos).astype(np.int64)
    pos_table = np.ascontiguousarray(np.asarray(pos_table), dtype=np.float32)

    wid = np.full((B, S), -1.0, np.float32)
    wrecw = np.zeros((B, W), np.float32)  # 1/len per word (0 for padding)
    lenw = np.zeros((B, W), np.float32)  # len per word (0 for padding)
    posoh = np.zeros((B, NPOS, W), np.float32)
    spans = []  # per sample: ([(g, t0, t1), ...], profile_key)
    for i in range(B):
        wl = word_lens[i]
        sl = int(seq_len[i])
        valid = wl != 0
        valid[0] = True
        ridx = np.nonzero(valid)[0]  # real words (contiguous prefix by construction)
        starts = wl[ridx]
        n = len(ridx)
        nxt = np.append(starts[1:], 0)
        ends = np.where(nxt == 0, sl, nxt)
        lens = np.maximum(ends - starts, 1)
        cidx = np.arange(sl)
        cwid = np.searchsorted(starts, cidx, side="right") - 1
        wid[i, :sl] = ridx[cwid].astype(np.float32)
        wrecw[i, ridx] = 1.0 / lens.astype(np.float32)
        lenw[i, ridx] = lens.astype(np.float32)
        posoh[i, pos[i], np.arange(W)] = 1.0  # one-hot part
        sp = []
        for g in range(NG):
            w0 = 128 * g
            if w0 >= n:
                continue
            w1 = min(128 * (g + 1), n)
            c0, c1 = int(starts[w0]), int(ends[w1 - 1])
            sp.append((g, c0 // 128, (c1 - 1) // 128))
        spans.append((sp, (n, sl)))

    assign = _assign_slots(spans)
    sched = []
    for s in range(SPC):
        u = np.zeros((NG, NT), bool)
        for i in assign[s]:
            for (g, t0, t1) in spans[i][0]:
                u[g, t0 : t1 + 1] = True
        sched.append(tuple(tuple(np.nonzero(u[g])[0].tolist()) for g in range(NG)))
    sched = tuple(sched)

    # [B, S] -> [B, 128, NT]: per-partition scalar columns per char tile
    widT = wid.reshape(B, NT, 128).transpose(0, 2, 1)
    # 1/len per word -> [B, 128, NG] per-partition scalars per word group
    wrecwT = wrecw.reshape(B, NG, 128).transpose(0, 2, 1)

    feats16 = feats.astype(np.float16)
    tab16 = pos_table.astype(np.float16)

    in_maps = []
    for c in range(N_CORES):
        cpb = np.zeros((64, CPB_W), np.float16)
        cpb[0:NPOS, CPB_TAB : CPB_TAB + D] = tab16
        cpr = np.zeros((128, CPR_W), np.float32)
        feats_c = np.empty((SPC, 128, NT * D), np.float16)
        for s in range(SPC):
            i = assign[s][c]
            # char c at partition c%128, cols (c//128)*D : (c//128+1)*D
            feats_c[s] = (
                feats16[i].reshape(NT, 128, D).transpose(1, 0, 2).reshape(128, NT * D)
            )
            cpr[:, CPR_META + NT * s : CPR_META + NT * (s + 1)] = widT[i]
            cpr[:, CPR_REC + NG * s : CPR_REC + NG * (s + 1)] = wrecwT[i]
            cpb[0:NPOS, CPB_POH + s * W : CPB_POH + (s + 1) * W] = (
                posoh[i] * lenw[i][None, :]
            ).astype(np.float16)
        in_maps.append({"feats": feats_c, "constpb": cpb, "constpr": cpr})
    return in_maps, sched, assign


def _run(in_maps, sched, assign, trace=False):
    from concourse.bass_utils import run_bass_kernel_spmd

    nc = _get_program(sched)
    res = run_bass_kernel_spmd(nc, in_maps, list(range(N_CORES)), trace=trace)
    out = np.empty((B, W, D), np.float32)
    for c in range(N_CORES):
        for s in range(SPC):
            # un-swizzle: word w at partition w%128, col block w//128
            o = res.results[c]["out"][s]  # [128, NG*D] fp16
            out[assign[s][c]] = (
                o.reshape(128, NG, D).transpose(1, 0, 2).reshape(W, D)
            ).astype(np.float32)
    return out, res


def kernel(feats, word_lens, seq_len, pos, pos_table):
    in_maps, sched, assign = _prep_inputs(feats, word_lens, seq_len, pos, pos_table)
    out, _ = _run(in_maps, sched, assign, trace=False)
    return out


# revision 16
# speedup vs baseline: 1.0793x; 1.0072x over previous
"""Trainium2 Bass kernel for CharModel ragged segment-mean + pos embedding.

Computation (per sample):
  out[j, :] = mean(feats[start_j:end_j, :]) * valid_j + pos_table[pos_j]
where the ragged segments are given by sorted word start offsets.

Strategy (fp16 data path; harness gate is rel_err < 2e-2 and fp16 lands
at ~7e-4, so no hi/lo compensation is needed):
  - Host precomputes per-char metadata: word_id[c] (which word each char
    belongs to, -1 for padding chars) and wrec[c] = 1/len(word(c)).
  - Device builds a one-hot matrix M[c, j] = (word_id[c]==j) with one
    fused DVE tensor_scalar op per 128-char tile, then the PE computes
    sum[W, D] = M.T @ feats directly in PSUM.  The pos embedding is
    added by accumulating (onehot(pos)*len).T @ pos_table into the same
    PSUM (both operands zero-padded to 128 contraction rows so the PE
    never switches tile quadrants), so the final 1/len multiply leaves
    mean + pos_table[pos].
  - feats ship pre-transposed in DRAM as [128, NT*D] fp16 (char c lives
    at partition c%128, tile c//128) so each per-sample feats DMA is
    per-partition contiguous: 128 descriptors instead of 128*NT, which
    cuts the HWDGE DIRECT2D issue cost ~4x.  The output is staged and
    written the same way ([128, NG*D] fp16, partition-contiguous) and
    un-swizzled on the host.
  - All DMAs ride the sync (HWDGE) queue with the metadata packs FIRST:
    SDMA rings drain in enqueue order, so const-after-feats would stall
    the warm-up + one-hot builds ~6us behind sample-0 feats data.
  - Data parallel over batch: 8 NeuronCores x 4 samples each, one shared
    SPMD program, per-core input maps.

Walrus ISA wait-slot limits dealt with throughout: matmul (S3_LW),
tensor_scalar (S3D3_TS) and DMA (PSEUDO_DMA_DIRECT2D) instructions can
carry only ONE semaphore wait each.  Hence:
  - the fp16 metadata (iota row + word-id scalars) ships in ONE packed
    tensor; the fp32 copy-scale reciprocals in a tiny second pack that a
    Scalar probe and a Vector probe observe once, so the PSUM->SBUF
    copies carry only their PE wait;
  - a per-sample 1x1 "gate" matmul (forced first in PE order via
    add_dep_helper, writing a dedicated never-overlapping PSUM region)
    alone carries the DVE wait for the lhsT builds and, transitively, the
    previous sample's PSUM bank releases;
  - feats tiles and output staging tiles get enough pool bufs that no
    slot is ever reused (no WAR waits on DMAs/copies);
  - two output DMAs per sample (SWDGE, each preceded by a Pool probe of
    the DVE-written half so the DMA carries only the ACT copy wait).
"""

import sys

if "/opt/trn_rl_repo" not in sys.path:
    sys.path.insert(0, "/opt/trn_rl_repo")

import numpy as np

import bass_rust
import concourse.bass as bass
import concourse.mybir as mybir
from concourse.tile import TileContext
from concourse.tile_sem_assignment import N_PROCS


class ChunkedDrainTileContext(TileContext):
    """TileContext whose kernel-tail drain is split into several drain
    instructions with one sem wait each (the CTRL_NO ISA struct rejects
    multi-wait drains here)."""

    DRAIN_CHUNK = 1

    def _drain_and_barrier(self, tick_clock, wait_clock):
        gc = tick_clock.global_clock
        ticks = [gc.peek_next(i) - 1 for i in range(N_PROCS)]
        active = [i for i, t in enumerate(ticks) if t > 0]
        for i in range(0, len(active), self.DRAIN_CHUNK):
            chunk = set(active[i : i + self.DRAIN_CHUNK])
            part = [ticks[j] if j in chunk else 0 for j in range(N_PROCS)]
            d = self.nc.sync.drain()
            wait_clock.add_sem_waits(
                d.ins, bass_rust.ScopedClock({None: bass_rust.VectorClock(part)})
            )
        self.nc.all_engine_barrier()
        assert self.sems is not None
        popped = self.nc._tile_sem_poison_stack.pop()
        assert popped is self._sem_poison
        self.nc.clear_and_free_semaphores(list(self.sems.allocated().values()))
        self.nc.all_engine_barrier()

B, S, D, W, NPOS = 32, 1024, 512 + 256, 512, 32  # D=768
N_CORES = 8
SPC = B // N_CORES  # samples per core
NT = S // 128  # char tiles per sample
NG = W // 128  # word groups per sample
CHUNKS = ((0, 384), (384, 384))  # D split for PSUM bank limit
F32 = mybir.dt.float32
F16 = mybir.dt.float16

# fp16 pos pack: only the 32 live rows ship over DMA; the zero rows
# 32:128 are memset on the DVE (shipping them would double the pack's
# critical-path DMA time)
CPB_POH = 0  # [32, SPC*W]: sample s cols s*W:(s+1)*W = onehot(pos)*len
CPB_TAB = SPC * W  # [32, D]: fp16(pos_table)
CPB_W = SPC * W + D

# fp32 pack (copy scales + word-id scalars; the is_equal build with f32
# stream + f32 scalar + fp16 out is the HW-proven config).  The iota row
# is generated on-device by the Pool engine, keeping this pack tiny so
# the one-hot builds start as early as possible.
CPR_REC = 0  # [128, NG*SPC]: per sample s: 1/len per word group
CPR_META = NG * SPC  # [128, NT*SPC]: per sample s: word-id per char
CPR_W = NG * SPC + NT * SPC


def _build_program(sched):
    """sched[s][g] = tuple of char-tile indices whose chars can touch word
    group g of slot-s samples on ANY core (union schedule; the one-hot
    lhsT zeroes contributions from tiles/words not actually present on a
    given core).  Matmuls for (g, t) pairs outside the schedule multiply
    all-zero one-hot slices and are skipped entirely."""
    nc = bass.Bass()
    # feats pre-transposed on host: char c of sample s lives at
    # [s, c % 128, (c // 128) * D : (c // 128 + 1) * D].
    feats = nc.declare_dram_parameter("feats", [SPC, 128, NT * D], F16, False)
    constpb = nc.declare_dram_parameter("constpb", [64, CPB_W], F16, False)
    constpr = nc.declare_dram_parameter("constpr", [128, CPR_W], F32, False)
    # output staged the same way: word w of sample s at
    # [s, w % 128, (w // 128) * D : ...]; host un-swizzles.
    out = nc.declare_dram_parameter("out", [SPC, 128, NG * D], F16, True)

    dep = lambda a, b, why: bass_rust.add_dep_helper(
        a.ins, b.ins, sync=False, reason=why
    )

    # per-sample used tile range (contiguous from 0 by construction)
    tmax = [max(t for g in range(NG) for t in sched[s][g]) for s in range(SPC)]
    with ChunkedDrainTileContext(nc) as tc:
        with (
            tc.tile_pool(name="const", bufs=1) as cpool,
            tc.tile_pool(name="feat", bufs=SPC) as fpool,
            tc.tile_pool(name="lhs", bufs=sum(t + 1 for t in tmax)) as lpool,
            tc.tile_pool(name="outsb", bufs=SPC) as opool,
            tc.tile_pool(name="psum", bufs=2 * NG, space="PSUM") as ppool,
        ):
            cpb = cpool.tile([128, CPB_W], F16)
            # iota row 0..W-1 on every partition, generated on-device
            # (fp32 is exact for these integers)
            iota_t = cpool.tile([128, W], F32)
            nc.gpsimd.iota(
                iota_t[:, :],
                pattern=[[1, W]],
                base=0,
                channel_multiplier=0,
                allow_small_or_imprecise_dtypes=True,
            )
            iota_f = iota_t[:, :]
            cpr = cpool.tile([128, CPR_W], F32)
            cpr_dma = nc.sync.dma_start(out=cpr[:, :], in_=constpr[:, :])
            # ACT probe: observe the f32-pack DMA on the Scalar engine once,
            # so the ACT PSUM->SBUF copies carry only their PE wait.  (The
            # DVE observes it via the first one-hot build.)  The DVE probe
            # observes the Pool iota tick so the builds carry only the
            # f32-pack wait.
            act_probe = cpool.tile([1, 2], F32)
            nc.scalar.copy(act_probe[0:1, 0:1], cpr[0:1, 0:1])
            nc.vector.tensor_copy(act_probe[0:1, 1:2], iota_t[0:1, 0:1])
            pl_probe = cpool.tile([1, 2 * SPC], F16)
            # cpb rows 64:128 zeroed on the Pool engine AFTER the iota (the
            # DMA ships rows 0:64, with 32:64 zero from the host); a tiny
            # "pgate" matmul below carries the memset tick for the pos
            # matmuls.
            nc.gpsimd.memset(cpb[64:128, :], 0.0)
            # Warm-up bank comes from the SAME pool as the matmul units:
            # sample 0's unit 7 recycles it with a free PE-order release.
            # All gates also write into pool tiles (their outputs are never
            # read; the units' start=True matmuls reset the banks), so all
            # 8 PSUM banks serve the matmul pipeline.
            warm = ppool.tile([128, 384], F32, tag="ps", name="warm")
            # PE warm-up: fat fp32 matmuls reading the on-device iota tile
            # run during the DMA ramp and trip the HAM clock gate before
            # the real matmuls start.  Without this the PE sometimes stays
            # at 1.2GHz for the whole kernel.
            for wi in range(5):
                nc.tensor.matmul(
                    warm[0:1, 0:320],
                    iota_t[:, 0:1],
                    iota_t[:, 0:320],
                    start=(wi == 0),
                    stop=(wi == 4),
                    skip_group_check=True,
                )
            # pgate: carries the Pool memset tick; every pos matmul deps on
            # it so none carries a second wait besides the cpb DMA sem.
            pgate = nc.tensor.matmul(
                warm[0:1, 0:1],
                cpb[64:65, 0:1],
                cpb[64:65, 0:1],
                start=True,
                stop=True,
                skip_group_check=True,
            )
            # feats DMAs: sample 0 in two pieces (so group-0/1 matmuls can
            # start as soon as their tiles land), later samples whole:
            # 7 sync-queue DMAs total, within the 8 HWDGE queues.
            fts = []
            for s in range(SPC):
                ft = fpool.tile([128, NT * D], F16, tag="ft", name=f"ft_{s}")
                fts.append(ft)
            # Input phasing: {cpr, s0} stream immediately; cpb starts once
            # the tiny f32 pack is in; {s1,s2,s3} start after the pos pack.
            # Each arrives just ahead of its consumer while the
            # head-critical packs get most of the bandwidth.
            ntile0 = tmax[0] + 1
            c1, c2 = min(3, ntile0 - 1), ntile0 - 1
            nc.sync.dma_start(out=fts[0][:, : c1 * D], in_=feats[0, :, : c1 * D])
            cpb_dma = nc.sync.dma_start(out=cpb[0:64, :], in_=constpb[:, :])
            bass_rust.add_dep_helper(
                cpb_dma.ins, cpr_dma.ins, reason="pos pack after f32 pack"
            )
            nc.sync.dma_start(
                out=fts[0][:, c1 * D : c2 * D], in_=feats[0, :, c1 * D : c2 * D]
            )
            s0_dma = nc.sync.dma_start(
                out=fts[0][:, c2 * D : ntile0 * D],
                in_=feats[0, :, c2 * D : ntile0 * D],
            )
            for s in range(1, SPC):
                fd = nc.sync.dma_start(
                    out=fts[s][:, : (tmax[s] + 1) * D],
                    in_=feats[s, :, : (tmax[s] + 1) * D],
                )
                bass_rust.add_dep_helper(
                    fd.ins, s0_dma.ins, reason="feats tail after sample 0"
                )

            def emit_builds(s):
                d = {}
                for t in range(tmax[s] + 1):
                    lh = lpool.tile([128, W], F16, tag="lh", name=f"lh_{s}_{t}")
                    wcol = CPR_META + NT * s
                    nc.vector.tensor_scalar(
                        lh[:, :],
                        iota_f,
                        cpr[:, wcol + t : wcol + t + 1],
                        None,
                        op0=mybir.AluOpType.is_equal,
                    )
                    d[t] = lh
                return d

            lhs_by_s = {0: emit_builds(0)}
            prev_ob = None  # previous sample's output staging buffer
            for s in range(SPC):
                ft = fts[s]
                last = s == SPC - 1
                last_dve_copy = None
                last_act_copy = None
                ntile = tmax[s] + 1
                lhs = lhs_by_s[s]
                gate = gate_g1 = gate2 = None
                bank_gates = []
                ob = opool.tile([128, NG * D], F16, tag="ob", name=f"ob_{s}")
                for g in range(NG):
                    if g == 2 and s + 1 < SPC:
                        # Emit the NEXT sample's one-hot builds here: they
                        # slot into the DVE queue between this sample's
                        # unit-2 and unit-4 copies, so they never sit behind
                        # a copy that is blocked on a late PSUM (DVE
                        # head-of-line), and the next gateA finds them done.
                        lhs_by_s[s + 1] = emit_builds(s + 1)
                    tiles_g = sched[s][g]
                    pss = [
                        ppool.tile([128, cn], F32, tag="ps", name=f"ps_{s}_{g}_{ci}")
                        for ci, (c0, cn) in enumerate(CHUNKS)
                    ]
                    # Bank gates at g0 and g2: each reads a prev-sample ob
                    # cell whose copy tick (same engine, same or later unit)
                    # also covers the release of the bank it writes, so one
                    # sem wait per gate -- and that tick is >= one group old
                    # by the time the PE gets here, so the PE never stalls
                    # at sample boundaries (stalls reset the HAM clock ramp
                    # and cost ~2x the gap).
                    if prev_ob is not None and g in (0, 2):
                        dcell = D if g == 0 else 3 * D
                        acell = D + 384 if g == 0 else NG * D - 1
                        gd = nc.tensor.matmul(
                            pss[0][0:1, 0:1],
                            prev_ob[0:1, dcell : dcell + 1],
                            prev_ob[0:1, dcell : dcell + 1],
                            start=True,
                            stop=True,
                            skip_group_check=True,
                        )
                        ga = nc.tensor.matmul(
                            pss[1][0:1, 0:1],
                            prev_ob[0:1, acell : acell + 1],
                            prev_ob[0:1, acell : acell + 1],
                            start=True,
                            stop=True,
                            skip_group_check=True,
                        )
                        bank_gates = [gd, ga]
                    if g == 0:
                        # build gate: carries the DVE wait for this sample's
                        # one-hot builds (group 0's tiles only for the
                        # head-critical sample 0)
                        tA = max(sched[s][0]) if s == 0 else ntile - 1
                        gate = nc.tensor.matmul(
                            pss[0][0:1, 1:2],
                            lhs[tA][0:1, 0:1],
                            lhs[tA][0:1, 0:1],
                            start=True,
                            stop=True,
                            skip_group_check=True,
                        )
                    if g == 1 and s == 0:
                        gate_g1 = nc.tensor.matmul(
                            pss[0][0:1, 1:2],
                            lhs[max(sched[s][1])][0:1, 0:1],
                            lhs[max(sched[s][1])][0:1, 0:1],
                            start=True,
                            stop=True,
                            skip_group_check=True,
                        )
                    if g == 2 and s == 0:
                        gate2 = nc.tensor.matmul(
                            pss[0][0:1, 1:2],
                            lhs[ntile - 1][0:1, 0:1],
                            lhs[ntile - 1][0:1, 0:1],
                            start=True,
                            stop=True,
                            skip_group_check=True,
                        )

                    def _mm_deps(mm):
                        dep(mm, gate, "matmuls after sample build gate")
                        if gate_g1 is not None and g == 1:
                            dep(mm, gate_g1, "matmuls after g1 build gate")
                        if gate2 is not None and g >= 2:
                            dep(mm, gate2, "matmuls after late build gate")
                        for bg in bank_gates:
                            dep(mm, bg, "matmuls after bank gate")

                    # chunk-inner order: the two matmuls of a (g, t) pair use
                    # the same stationary weights back to back
                    for k, t in enumerate(tiles_g):
                        for ci, (c0, cn) in enumerate(CHUNKS):
                            mm = nc.tensor.matmul(
                                pss[ci][:, :],
                                lhs[t][:, 128 * g : 128 * (g + 1)],
                                ft[:, t * D + c0 : t * D + c0 + cn],
                                start=(k == 0),
                                stop=False,
                                skip_group_check=True,
                            )
                            _mm_deps(mm)
                    # pos contribution scaled by len (exact in fp16) so the
                    # final 1/len multiply leaves pos_table[pos] exactly;
                    # both operands are zero-padded to 128 rows
                    pcol = CPB_POH + s * W
                    for ci, (c0, cn) in enumerate(CHUNKS):
                        mm = nc.tensor.matmul(
                            pss[ci][:, :],
                            cpb[:, pcol + 128 * g : pcol + 128 * (g + 1)],
                            cpb[:, CPB_TAB + c0 : CPB_TAB + c0 + cn],
                            start=(len(tiles_g) == 0),
                            stop=True,
                            skip_group_check=True,
                        )
                        dep(mm, pgate, "pos matmul after memset gate")
                        _mm_deps(mm)
                    for ci, (c0, cn) in enumerate(CHUNKS):
                        unit = 2 * g + ci
                        recip_ap = cpr[
                            :, CPR_REC + NG * s + g : CPR_REC + NG * s + g + 1
                        ]
                        # last sample: units 0-3 all on ACT so the g1-point
                        # output DMA needs only the chained unit-3 ACT wait
                        use_dve = (unit % 2 == 0) and not (last and unit < 4)
                        if use_dve:
                            cp = nc.vector.tensor_scalar(
                                ob[:, g * D + c0 : g * D + c0 + cn],
                                pss[ci][:, :],
                                recip_ap,
                                None,
                                op0=mybir.AluOpType.mult,
                            )
                            if last_dve_copy is not None:
                                dep(cp, last_dve_copy, "DVE copy order")
                            last_dve_copy = cp
                        else:
                            cp = nc.scalar.activation(
                                ob[:, g * D + c0 : g * D + c0 + cn],
                                pss[ci][:, :],
                                mybir.ActivationFunctionType.Copy,
                                scale=recip_ap,
                            )
                            if last_act_copy is not None:
                                dep(cp, last_act_copy, "ACT copy order")
                            last_act_copy = cp
                    # streaming output: sample 0 ships once at the end
                    # (it is never tail-critical); middle samples split at
                    # the g2 point; the last sample ships three pieces so
                    # most of its output flows during its own compute.
                    # Pool probes observe the DVE copy chain so each DMA
                    # carries only the chained ACT wait.
                    if last and g == 1:
                        # units 0-3 are all ACT copies: the DMA's unit-3
                        # wait covers them by the ACT dep chain
                        nc.gpsimd.dma_start(
                            out=out[s, :, : 2 * D], in_=ob[:, : 2 * D]
                        )
                    if g == 2 and s != 0:
                        lo = 2 * D if last else 0
                        nc.gpsimd.tensor_copy(
                            pl_probe[0:1, 2 * s : 2 * s + 1],
                            ob[0:1, 2 * D : 2 * D + 1],
                        )
                        nc.gpsimd.dma_start(
                            out=out[s, :, lo : 3 * D], in_=ob[:, lo : 3 * D]
                        )
                # Pool probe: observe the last DVE copy's tick on the Pool
                # engine so the output DMA carries only the ACT copy wait.
                nc.gpsimd.tensor_copy(
                    pl_probe[0:1, 2 * s + 1 : 2 * s + 2], ob[0:1, 3 * D : 3 * D + 1]
                )
                lo = 3 * D if s != 0 else 0
                nc.gpsimd.dma_start(out=out[s, :, lo:], in_=ob[:, lo:])
                prev_ob = ob
    return nc


_PROGRAM_CACHE = {}


def _get_program(sched):
    key = tuple(tuple(tuple(g) for g in s) for s in sched)
    if key not in _PROGRAM_CACHE:
        _PROGRAM_CACHE[key] = _build_program(sched)
    return _PROGRAM_CACHE[key]


def _assign_slots(spans):
    """Assign the B samples to (slot, core) so that the per-slot UNION of
    (group, char-tile) matmul footprints is small: sort by profile, then
    cheap local-search swaps."""
    import random

    def union_cost(assign):
        total = 0
        for slot in assign:
            u = np.zeros((NG, NT), bool)
            for i in slot:
                for (g, t0, t1) in spans[i][0]:
                    u[g, t0 : t1 + 1] = True
            total += int(u.sum())
        return total

    order = sorted(range(B), key=lambda i: spans[i][1])
    best_assign, best = None, None
    for seed in range(6):
        rng = random.Random(seed)
        assign = [
            [order[s * N_CORES + c] for c in range(N_CORES)] for s in range(SPC)
        ]
        if seed:
            flat = [i for slot in assign for i in slot]
            rng.shuffle(flat)
            assign = [flat[s * N_CORES : (s + 1) * N_CORES] for s in range(SPC)]
        cost = union_cost(assign)
        for _ in range(30000):
            s1, s2 = rng.randrange(SPC), rng.randrange(SPC)
            if s1 == s2:
                continue
            i1, i2 = rng.randrange(N_CORES), rng.randrange(N_CORES)
            assign[s1][i1], assign[s2][i2] = assign[s2][i2], assign[s1][i1]
            c = union_cost(assign)
            if c <= cost:
                cost = c
            else:
                assign[s1][i1], assign[s2][i2] = assign[s2][i2], assign[s1][i1]
        if best is None or cost < best:
            best, best_assign = cost, [list(s) for s in assign]
    return best_assign


def _prep_inputs(feats, word_lens, seq_len, pos, pos_table):
    """Host-side metadata prep + batch sharding -> per-core input maps,
    union matmul schedule, and the sample->(slot, core) assignment."""
    feats = np.ascontiguousarray(np.asarray(feats), dtype=np.float32)
    word_lens = np.asarray(word_lens).astype(np.int64)
    seq_len = np.asarray(seq_len).astype(np.int64)
    pos = np.asarray(pos).astype(np.int64)
    pos_table = np.ascontiguousarray(np.asarray(pos_table), dtype=np.float32)

    wid = np.full((B, S), -1.0, np.float32)
    wrecw = np.zeros((B, W), np.float32)  # 1/len per word (0 for padding)
    lenw = np.zeros((B, W), np.float32)  # len per word (0 for padding)
    posoh = np.zeros((B, NPOS, W), np.float32)
    spans = []  # per sample: ([(g, t0, t1), ...], profile_key)
    for i in range(B):
        wl = word_lens[i]
        sl = int(seq_len[i])
        valid = wl != 0
        valid[0] = True
        ridx = np.nonzero(valid)[0]  # real words (contiguous prefix by construction)
        starts = wl[ridx]
        n = len(ridx)
        nxt = np.append(starts[1:], 0)
        ends = np.where(nxt == 0, sl, nxt)
        lens = np.maximum(ends - starts, 1)
        cidx = np.arange(sl)
        cwid = np.searchsorted(starts, cidx, side="right") - 1
        wid[i, :sl] = ridx[cwid].astype(np.float32)
        wrecw[i, ridx] = 1.0 / lens.astype(np.float32)
        lenw[i, ridx] = lens.astype(np.float32)
        posoh[i, pos[i], np.arange(W)] = 1.0  # one-hot part
        sp = []
        for g in range(NG):
            w0 = 128 * g
            if w0 >= n:
                continue
            w1 = min(128 * (g + 1), n)
            c0, c1 = int(starts[w0]), int(ends[w1 - 1])
            sp.append((g, c0 // 128, (c1 - 1) // 128))
        spans.append((sp, (n, sl)))

    assign = _assign_slots(spans)
    sched = []
    for s in range(SPC):
        u = np.zeros((NG, NT), bool)
        for i in assign[s]:
            for (g, t0, t1) in spans[i][0]:
                u[g, t0 : t1 + 1] = True
        sched.append(tuple(tuple(np.nonzero(u[g])[0].tolist()) for g in range(NG)))
    sched = tuple(sched)

    # [B, S] -> [B, 128, NT]: per-partition scalar columns per char tile
    widT = wid.reshape(B, NT, 128).transpose(0, 2, 1)
    # 1/len per word -> [B, 128, NG] per-partition scalars per word group
    wrecwT = wrecw.reshape(B, NG, 128).transpose(0, 2, 1)

    feats16 = feats.astype(np.float16)
    tab16 = pos_table.astype(np.float16)

    in_maps = []
    for c in range(N_CORES):
        cpb = np.zeros((64, CPB_W), np.float16)
        cpb[0:NPOS, CPB_TAB : CPB_TAB + D] = tab16
        cpr = np.zeros((128, CPR_W), np.float32)
        feats_c = np.empty((SPC, 128, NT * D), np.float16)
        for s in range(SPC):
            i = assign[s][c]
            # char c at partition c%128, cols (c//128)*D : (c//128+1)*D
            feats_c[s] = (
                feats16[i].reshape(NT, 128, D).transpose(1, 0, 2).reshape(128, NT * D)
            )
            cpr[:, CPR_META + NT * s : CPR_META + NT * (s + 1)] = widT[i]
            cpr[:, CPR_REC + NG * s : CPR_REC + NG * (s + 1)] = wrecwT[i]
            cpb[0:NPOS, CPB_POH + s * W : CPB_POH + (s + 1) * W] = (
                posoh[i] * lenw[i][None, :]
            ).astype(np.float16)
        in_maps.append({"feats": feats_c, "constpb": cpb, "constpr": cpr})
    return in_maps, sched, assign


def _run(in_maps, sched, assign, trace=False):
    from concourse.bass_utils import run_bass_kernel_spmd

    nc = _get_program(sched)
    res = run_bass_kernel_spmd(nc, in_maps, list(range(N_CORES)), trace=trace)
    out = np.empty((B, W, D), np.float32)
    for c in range(N_CORES):
        for s in range(SPC):
            # un-swizzle: word w at partition w%128, col block w//128
            o = res.results[c]["out"][s]  # [128, NG*D] fp16
            out[assign[s][c]] = (
                o.reshape(128, NG, D).transpose(1, 0, 2).reshape(W, D)
            ).astype(np.float32)
    return out, res


def kernel(feats, word_lens, seq_len, pos, pos_table):
    in_maps, sched, assign = _prep_inputs(feats, word_lens, seq_len, pos, pos_table)
    out, _ = _run(in_maps, sched, assign, trace=False)
    return out


# revision 17
# speedup vs baseline: 1.1008x; 1.0200x over previous
"""Trainium2 Bass kernel for CharModel ragged segment-mean + pos embedding.

Computation (per sample):
  out[j, :] = mean(feats[start_j:end_j, :]) * valid_j + pos_table[pos_j]
where the ragged segments are given by sorted word start offsets.

Strategy (fp16 data path; harness gate is rel_err < 2e-2 and fp16 lands
at ~7e-4, so no hi/lo compensation is needed):
  - Host precomputes per-char metadata: word_id[c] (which word each char
    belongs to, -1 for padding chars) and wrec[c] = 1/len(word(c)).
  - Device builds a one-hot matrix M[c, j] = (word_id[c]==j) with one
    fused DVE tensor_scalar op per 128-char tile, then the PE computes
    sum[W, D] = M.T @ feats directly in PSUM.  The pos embedding is
    added by accumulating (onehot(pos)*len).T @ pos_table into the same
    PSUM (both operands zero-padded to 128 contraction rows so the PE
    never switches tile quadrants), so the final 1/len multiply leaves
    mean + pos_table[pos].
  - feats ship pre-transposed in DRAM as [128, NT*D] fp16 (char c lives
    at partition c%128, tile c//128) so each per-sample feats DMA is
    per-partition contiguous: 128 descriptors instead of 128*NT, which
    cuts the HWDGE DIRECT2D issue cost ~4x.  The output is staged and
    written the same way ([128, NG*D] fp16, partition-contiguous) and
    un-swizzled on the host.
  - All DMAs ride the sync (HWDGE) queue with the metadata packs FIRST:
    SDMA rings drain in enqueue order, so const-after-feats would stall
    the warm-up + one-hot builds ~6us behind sample-0 feats data.
  - Data parallel over batch: 8 NeuronCores x 4 samples each, one shared
    SPMD program, per-core input maps.

Walrus ISA wait-slot limits dealt with throughout: matmul (S3_LW),
tensor_scalar (S3D3_TS) and DMA (PSEUDO_DMA_DIRECT2D) instructions can
carry only ONE semaphore wait each.  Hence:
  - the fp16 metadata (iota row + word-id scalars) ships in ONE packed
    tensor; the fp32 copy-scale reciprocals in a tiny second pack that a
    Scalar probe and a Vector probe observe once, so the PSUM->SBUF
    copies carry only their PE wait;
  - a per-sample 1x1 "gate" matmul (forced first in PE order via
    add_dep_helper, writing a dedicated never-overlapping PSUM region)
    alone carries the DVE wait for the lhsT builds and, transitively, the
    previous sample's PSUM bank releases;
  - feats tiles and output staging tiles get enough pool bufs that no
    slot is ever reused (no WAR waits on DMAs/copies);
  - two output DMAs per sample (SWDGE, each preceded by a Pool probe of
    the DVE-written half so the DMA carries only the ACT copy wait).
"""

import sys

if "/opt/trn_rl_repo" not in sys.path:
    sys.path.insert(0, "/opt/trn_rl_repo")

import numpy as np

import bass_rust
import concourse.bass as bass
import concourse.mybir as mybir
from concourse.tile import TileContext
from concourse.tile_sem_assignment import N_PROCS


class ChunkedDrainTileContext(TileContext):
    """TileContext whose kernel-tail drain is split into several drain
    instructions with one sem wait each (the CTRL_NO ISA struct rejects
    multi-wait drains here)."""

    DRAIN_CHUNK = 1

    def _drain_and_barrier(self, tick_clock, wait_clock):
        gc = tick_clock.global_clock
        ticks = [gc.peek_next(i) - 1 for i in range(N_PROCS)]
        active = [i for i, t in enumerate(ticks) if t > 0]
        for i in range(0, len(active), self.DRAIN_CHUNK):
            chunk = set(active[i : i + self.DRAIN_CHUNK])
            part = [ticks[j] if j in chunk else 0 for j in range(N_PROCS)]
            d = self.nc.sync.drain()
            wait_clock.add_sem_waits(
                d.ins, bass_rust.ScopedClock({None: bass_rust.VectorClock(part)})
            )
        self.nc.all_engine_barrier()
        assert self.sems is not None
        popped = self.nc._tile_sem_poison_stack.pop()
        assert popped is self._sem_poison
        self.nc.clear_and_free_semaphores(list(self.sems.allocated().values()))
        self.nc.all_engine_barrier()

B, S, D, W, NPOS = 32, 1024, 512 + 256, 512, 32  # D=768
N_CORES = 8
SPC = B // N_CORES  # samples per core
NT = S // 128  # char tiles per sample
NG = W // 128  # word groups per sample
CHUNKS = ((0, 384), (384, 384))  # D split for PSUM bank limit
F32 = mybir.dt.float32
F16 = mybir.dt.float16

# fp16 pos pack: only the 32 live rows ship over DMA; the zero rows
# 32:128 are memset on the DVE (shipping them would double the pack's
# critical-path DMA time)
CPB_POH = 0  # [32, SPC*W]: sample s cols s*W:(s+1)*W = onehot(pos)*len
CPB_TAB = SPC * W  # [32, D]: fp16(pos_table)
CPB_W = SPC * W + D

# fp32 pack (copy scales + word-id scalars; the is_equal build with f32
# stream + f32 scalar + fp16 out is the HW-proven config).  The iota row
# is generated on-device by the Pool engine, keeping this pack tiny so
# the one-hot builds start as early as possible.
CPR_REC = 0  # [128, NG*SPC]: per sample s: 1/len per word group
CPR_META = NG * SPC  # [128, NT*SPC]: per sample s: word-id per char
CPR_W = NG * SPC + NT * SPC


def _build_program(sched):
    """sched[s][g] = tuple of char-tile indices whose chars can touch word
    group g of slot-s samples on ANY core (union schedule; the one-hot
    lhsT zeroes contributions from tiles/words not actually present on a
    given core).  Matmuls for (g, t) pairs outside the schedule multiply
    all-zero one-hot slices and are skipped entirely."""
    nc = bass.Bass()
    # feats pre-transposed on host: char c of sample s lives at
    # [s, c % 128, (c // 128) * D : (c // 128 + 1) * D].
    feats = nc.declare_dram_parameter("feats", [SPC, 128, NT * D], F16, False)
    constpb = nc.declare_dram_parameter("constpb", [64, CPB_W], F16, False)
    constpr = nc.declare_dram_parameter("constpr", [128, CPR_W], F32, False)
    # output staged the same way: word w of sample s at
    # [s, w % 128, (w // 128) * D : ...]; host un-swizzles.
    out = nc.declare_dram_parameter("out", [SPC, 128, NG * D], F16, True)

    dep = lambda a, b, why: bass_rust.add_dep_helper(
        a.ins, b.ins, sync=False, reason=why
    )

    # per-sample used tile range (contiguous from 0 by construction)
    tmax = [max(t for g in range(NG) for t in sched[s][g]) for s in range(SPC)]
    with ChunkedDrainTileContext(nc) as tc:
        with (
            tc.tile_pool(name="const", bufs=1) as cpool,
            tc.tile_pool(name="feat", bufs=SPC) as fpool,
            tc.tile_pool(name="lhs", bufs=sum(t + 1 for t in tmax)) as lpool,
            tc.tile_pool(name="outsb", bufs=SPC) as opool,
            tc.tile_pool(name="psum", bufs=2 * NG, space="PSUM") as ppool,
        ):
            cpb = cpool.tile([128, CPB_W], F16)
            # iota row 0..W-1 on every partition, generated on-device
            # (fp32 is exact for these integers)
            iota_t = cpool.tile([128, W], F32)
            nc.gpsimd.iota(
                iota_t[:, :],
                pattern=[[1, W]],
                base=0,
                channel_multiplier=0,
                allow_small_or_imprecise_dtypes=True,
            )
            iota_f = iota_t[:, :]
            cpr = cpool.tile([128, CPR_W], F32)
            cpr_dma = nc.sync.dma_start(out=cpr[:, :], in_=constpr[:, :])
            # ACT probe: observe the f32-pack DMA on the Scalar engine once,
            # so the ACT PSUM->SBUF copies carry only their PE wait.  (The
            # DVE observes it via the first one-hot build.)  The DVE probe
            # observes the Pool iota tick so the builds carry only the
            # f32-pack wait.
            act_probe = cpool.tile([1, 2], F32)
            nc.scalar.copy(act_probe[0:1, 0:1], cpr[0:1, 0:1])
            nc.vector.tensor_copy(act_probe[0:1, 1:2], iota_t[0:1, 0:1])
            pl_probe = cpool.tile([1, 2 * SPC], F16)
            # cpb rows 64:128 zeroed on the Pool engine AFTER the iota (the
            # DMA ships rows 0:64, with 32:64 zero from the host); a tiny
            # "pgate" matmul below carries the memset tick for the pos
            # matmuls.
            nc.gpsimd.memset(cpb[64:128, :], 0.0)
            # Warm-up bank comes from the SAME pool as the matmul units:
            # sample 0's unit 7 recycles it with a free PE-order release.
            # All gates also write into pool tiles (their outputs are never
            # read; the units' start=True matmuls reset the banks), so all
            # 8 PSUM banks serve the matmul pipeline.
            warm = ppool.tile([128, 384], F32, tag="ps", name="warm")
            # PE warm-up: fat fp32 matmuls reading the on-device iota tile
            # run during the DMA ramp and trip the HAM clock gate before
            # the real matmuls start.  Without this the PE sometimes stays
            # at 1.2GHz for the whole kernel.
            for wi in range(5):
                nc.tensor.matmul(
                    warm[0:1, 0:320],
                    iota_t[:, 0:1],
                    iota_t[:, 0:320],
                    start=(wi == 0),
                    stop=(wi == 4),
                    skip_group_check=True,
                )
            # pgate: carries the Pool memset tick; every pos matmul deps on
            # it so none carries a second wait besides the cpb DMA sem.
            pgate = nc.tensor.matmul(
                warm[0:1, 0:1],
                cpb[64:65, 0:1],
                cpb[64:65, 0:1],
                start=True,
                stop=True,
                skip_group_check=True,
            )
            # feats DMAs: sample 0 in two pieces (so group-0/1 matmuls can
            # start as soon as their tiles land), later samples whole:
            # 7 sync-queue DMAs total, within the 8 HWDGE queues.
            fts = []
            for s in range(SPC):
                ft = fpool.tile([128, NT * D], F16, tag="ft", name=f"ft_{s}")
                fts.append(ft)
            # Input phasing (HWDGE queues bandwidth-share, so ordering
            # only comes from completion-sem deps; each hop costs ~1.4us
            # of issue+straggler, so the chain keeps ~2 transfers in
            # flight and sizes pieces to land just-in-time): cpr and the
            # group-0/1 tiles of sample 0 stream first, everything else
            # chains pairwise behind them.
            ntile0 = tmax[0] + 1
            c1 = min(max(max(sched[0][0]), max(sched[0][1])) + 1, ntile0)
            s0a = nc.sync.dma_start(
                out=fts[0][:, : c1 * D], in_=feats[0, :, : c1 * D]
            )
            cpb_dma = nc.sync.dma_start(out=cpb[0:64, :], in_=constpb[:, :])
            bass_rust.add_dep_helper(
                cpb_dma.ins, cpr_dma.ins, reason="pos pack after f32 pack"
            )
            prev_a, prev_b = s0a, None
            if c1 < ntile0:
                s0b = nc.sync.dma_start(
                    out=fts[0][:, c1 * D : ntile0 * D],
                    in_=feats[0, :, c1 * D : ntile0 * D],
                )
                bass_rust.add_dep_helper(
                    s0b.ins, cpb_dma.ins, reason="s0 tail after pos pack"
                )
                prev_b = s0b
            # sample 1 in halves, samples 2/3 whole, chained pairwise
            n1 = tmax[1] + 1
            h1 = (n1 + 1) // 2
            s1a = nc.sync.dma_start(
                out=fts[1][:, : h1 * D], in_=feats[1, :, : h1 * D]
            )
            bass_rust.add_dep_helper(s1a.ins, prev_a.ins, reason="chain")
            s1b = nc.sync.dma_start(
                out=fts[1][:, h1 * D : n1 * D], in_=feats[1, :, h1 * D : n1 * D]
            )
            bass_rust.add_dep_helper(
                s1b.ins, (prev_b or prev_a).ins, reason="chain"
            )
            fd = nc.sync.dma_start(
                out=fts[2][:, : (tmax[2] + 1) * D],
                in_=feats[2, :, : (tmax[2] + 1) * D],
            )
            bass_rust.add_dep_helper(fd.ins, s1a.ins, reason="chain")
            fd = nc.sync.dma_start(
                out=fts[3][:, : (tmax[3] + 1) * D],
                in_=feats[3, :, : (tmax[3] + 1) * D],
            )
            bass_rust.add_dep_helper(fd.ins, s1b.ins, reason="chain")

            def emit_builds(s):
                d = {}
                for t in range(tmax[s] + 1):
                    lh = lpool.tile([128, W], F16, tag="lh", name=f"lh_{s}_{t}")
                    wcol = CPR_META + NT * s
                    nc.vector.tensor_scalar(
                        lh[:, :],
                        iota_f,
                        cpr[:, wcol + t : wcol + t + 1],
                        None,
                        op0=mybir.AluOpType.is_equal,
                    )
                    d[t] = lh
                return d

            lhs_by_s = {0: emit_builds(0)}
            prev_ob = None  # previous sample's output staging buffer
            for s in range(SPC):
                ft = fts[s]
                last = s == SPC - 1
                last_dve_copy = None
                last_act_copy = None
                ntile = tmax[s] + 1
                lhs = lhs_by_s[s]
                gate = gate2 = None
                bank_gates = []
                ob = opool.tile([128, NG * D], F16, tag="ob", name=f"ob_{s}")
                for g in range(NG):
                    if g == 2 and s + 1 < SPC:
                        # Emit the NEXT sample's one-hot builds here: they
                        # slot into the DVE queue between this sample's
                        # unit-2 and unit-4 copies, so they never sit behind
                        # a copy that is blocked on a late PSUM (DVE
                        # head-of-line), and the next gateA finds them done.
                        lhs_by_s[s + 1] = emit_builds(s + 1)
                    tiles_g = sched[s][g]
                    pss = [
                        ppool.tile([128, cn], F32, tag="ps", name=f"ps_{s}_{g}_{ci}")
                        for ci, (c0, cn) in enumerate(CHUNKS)
                    ]
                    # Bank gates at g0 and g2: each reads a prev-sample ob
                    # cell whose copy tick (same engine, same or later unit)
                    # also covers the release of the bank it writes, so one
                    # sem wait per gate -- and that tick is >= one group old
                    # by the time the PE gets here, so the PE never stalls
                    # at sample boundaries (stalls reset the HAM clock ramp
                    # and cost ~2x the gap).
                    if prev_ob is not None and g in (0, 2):
                        dcell = D if g == 0 else 3 * D
                        acell = D + 384 if g == 0 else NG * D - 1
                        gd = nc.tensor.matmul(
                            pss[0][0:1, 0:1],
                            prev_ob[0:1, dcell : dcell + 1],
                            prev_ob[0:1, dcell : dcell + 1],
                            start=True,
                            stop=True,
                            skip_group_check=True,
                        )
                        ga = nc.tensor.matmul(
                            pss[1][0:1, 0:1],
                            prev_ob[0:1, acell : acell + 1],
                            prev_ob[0:1, acell : acell + 1],
                            start=True,
                            stop=True,
                            skip_group_check=True,
                        )
                        bank_gates = [gd, ga]
                    if g == 0:
                        # build gate: carries the DVE wait for this sample's
                        # one-hot builds (group 0's tiles only for the
                        # head-critical sample 0)
                        tA = (
                            max(max(sched[s][0]), max(sched[s][1]))
                            if s == 0
                            else ntile - 1
                        )
                        gate = nc.tensor.matmul(
                            pss[0][0:1, 1:2],
                            lhs[tA][0:1, 0:1],
                            lhs[tA][0:1, 0:1],
                            start=True,
                            stop=True,
                            skip_group_check=True,
                        )
                    if g == 2 and s == 0:
                        gate2 = nc.tensor.matmul(
                            pss[0][0:1, 1:2],
                            lhs[ntile - 1][0:1, 0:1],
                            lhs[ntile - 1][0:1, 0:1],
                            start=True,
                            stop=True,
                            skip_group_check=True,
                        )

                    def _mm_deps(mm):
                        dep(mm, gate, "matmuls after sample build gate")
                        if gate2 is not None and g >= 2:
                            dep(mm, gate2, "matmuls after late build gate")
                        for bg in bank_gates:
                            dep(mm, bg, "matmuls after bank gate")

                    # chunk-inner order: the two matmuls of a (g, t) pair use
                    # the same stationary weights back to back
                    for k, t in enumerate(tiles_g):
                        for ci, (c0, cn) in enumerate(CHUNKS):
                            mm = nc.tensor.matmul(
                                pss[ci][:, :],
                                lhs[t][:, 128 * g : 128 * (g + 1)],
                                ft[:, t * D + c0 : t * D + c0 + cn],
                                start=(k == 0),
                                stop=False,
                                skip_group_check=True,
                            )
                            _mm_deps(mm)
                    # pos contribution scaled by len (exact in fp16) so the
                    # final 1/len multiply leaves pos_table[pos] exactly;
                    # both operands are zero-padded to 128 rows
                    pcol = CPB_POH + s * W
                    for ci, (c0, cn) in enumerate(CHUNKS):
                        mm = nc.tensor.matmul(
                            pss[ci][:, :],
                            cpb[:, pcol + 128 * g : pcol + 128 * (g + 1)],
                            cpb[:, CPB_TAB + c0 : CPB_TAB + c0 + cn],
                            start=(len(tiles_g) == 0),
                            stop=True,
                            skip_group_check=True,
                        )
                        dep(mm, pgate, "pos matmul after memset gate")
                        _mm_deps(mm)
                    for ci, (c0, cn) in enumerate(CHUNKS):
                        unit = 2 * g + ci
                        recip_ap = cpr[
                            :, CPR_REC + NG * s + g : CPR_REC + NG * s + g + 1
                        ]
                        # last sample: units 0-3 all on ACT so the g1-point
                        # output DMA needs only the chained unit-3 ACT wait
                        use_dve = (unit % 2 == 0) and not (last and unit < 4)
                        if use_dve:
                            cp = nc.vector.tensor_scalar(
                                ob[:, g * D + c0 : g * D + c0 + cn],
                                pss[ci][:, :],
                                recip_ap,
                                None,
                                op0=mybir.AluOpType.mult,
                            )
                            if last_dve_copy is not None:
                                dep(cp, last_dve_copy, "DVE copy order")
                            last_dve_copy = cp
                        else:
                            cp = nc.scalar.activation(
                                ob[:, g * D + c0 : g * D + c0 + cn],
                                pss[ci][:, :],
                                mybir.ActivationFunctionType.Copy,
                                scale=recip_ap,
                            )
                            if last_act_copy is not None:
                                dep(cp, last_act_copy, "ACT copy order")
                            last_act_copy = cp
                    # streaming output: sample 0 ships once at the end
                    # (it is never tail-critical); middle samples split at
                    # the g2 point; the last sample ships three pieces so
                    # most of its output flows during its own compute.
                    # Pool probes observe the DVE copy chain so each DMA
                    # carries only the chained ACT wait.
                    if last and g == 1:
                        # units 0-3 are all ACT copies: the DMA's unit-3
                        # wait covers them by the ACT dep chain
                        nc.gpsimd.dma_start(
                            out=out[s, :, : 2 * D], in_=ob[:, : 2 * D]
                        )
                    if g == 2 and s != 0:
                        lo = 2 * D if last else 0
                        nc.gpsimd.tensor_copy(
                            pl_probe[0:1, 2 * s : 2 * s + 1],
                            ob[0:1, 2 * D : 2 * D + 1],
                        )
                        nc.gpsimd.dma_start(
                            out=out[s, :, lo : 3 * D], in_=ob[:, lo : 3 * D]
                        )
                # Pool probe: observe the last DVE copy's tick on the Pool
                # engine so the output DMA carries only the ACT copy wait.
                nc.gpsimd.tensor_copy(
                    pl_probe[0:1, 2 * s + 1 : 2 * s + 2], ob[0:1, 3 * D : 3 * D + 1]
                )
                lo = 3 * D if s != 0 else 0
                nc.gpsimd.dma_start(out=out[s, :, lo:], in_=ob[:, lo:])
                prev_ob = ob
    return nc


_PROGRAM_CACHE = {}


def _get_program(sched):
    key = tuple(tuple(tuple(g) for g in s) for s in sched)
    if key not in _PROGRAM_CACHE:
        _PROGRAM_CACHE[key] = _build_program(sched)
    return _PROGRAM_CACHE[key]


def _assign_slots(spans):
    """Assign the B samples to (slot, core) so that the per-slot UNION of
    (group, char-tile) matmul footprints is small: sort by profile, then
    cheap local-search swaps."""
    import random

    def union_cost(assign):
        total = 0
        for slot in assign:
            u = np.zeros((NG, NT), bool)
            for i in slot:
                for (g, t0, t1) in spans[i][0]:
                    u[g, t0 : t1 + 1] = True
            total += int(u.sum())
        return total

    order = sorted(range(B), key=lambda i: spans[i][1])
    best_assign, best = None, None
    for seed in range(6):
        rng = random.Random(seed)
        assign = [
            [order[s * N_CORES + c] for c in range(N_CORES)] for s in range(SPC)
        ]
        if seed:
            flat = [i for slot in assign for i in slot]
            rng.shuffle(flat)
            assign = [flat[s * N_CORES : (s + 1) * N_CORES] for s in range(SPC)]
        cost = union_cost(assign)
        for _ in range(30000):
            s1, s2 = rng.randrange(SPC), rng.randrange(SPC)
            if s1 == s2:
                continue
            i1, i2 = rng.randrange(N_CORES), rng.randrange(N_CORES)
            assign[s1][i1], assign[s2][i2] = assign[s2][i2], assign[s1][i1]
            c = union_cost(assign)
            if c <= cost:
                cost = c
            else:
                assign[s1][i1], assign[s2][i2] = assign[s2][i2], assign[s1][i1]
        if best is None or cost < best:
            best, best_assign = cost, [list(s) for s in assign]
    return best_assign


def _prep_inputs(feats, word_lens, seq_len, pos, pos_table):
    """Host-side metadata prep + batch sharding -> per-core input maps,
    union matmul schedule, and the sample->(slot, core) assignment."""
    feats = np.ascontiguousarray(np.asarray(feats), dtype=np.float32)
    word_lens = np.asarray(word_lens).astype(np.int64)
    seq_len = np.asarray(seq_len).astype(np.int64)
    pos = np.asarray(pos).astype(np.int64)
    pos_table = np.ascontiguousarray(np.asarray(pos_table), dtype=np.float32)

    wid = np.full((B, S), -1.0, np.float32)
    wrecw = np.zeros((B, W), np.float32)  # 1/len per word (0 for padding)
    lenw = np.zeros((B, W), np.float32)  # len per word (0 for padding)
    posoh = np.zeros((B, NPOS, W), np.float32)
    spans = []  # per sample: ([(g, t0, t1), ...], profile_key)
    for i in range(B):
        wl = word_lens[i]
        sl = int(seq_len[i])
        valid = wl != 0
        valid[0] = True
        ridx = np.nonzero(valid)[0]  # real words (contiguous prefix by construction)
        starts = wl[ridx]
        n = len(ridx)
        nxt = np.append(starts[1:], 0)
        ends = np.where(nxt == 0, sl, nxt)
        lens = np.maximum(ends - starts, 1)
        cidx = np.arange(sl)
        cwid = np.searchsorted(starts, cidx, side="right") - 1
        wid[i, :sl] = ridx[cwid].astype(np.float32)
        wrecw[i, ridx] = 1.0 / lens.astype(np.float32)
        lenw[i, ridx] = lens.astype(np.float32)
        posoh[i, pos[i], np.arange(W)] = 1.0  # one-hot part
        sp = []
        for g in range(NG):
            w0 = 128 * g
            if w0 >= n:
                continue
            w1 = min(128 * (g + 1), n)
            c0, c1 = int(starts[w0]), int(ends[w1 - 1])
            sp.append((g, c0 // 128, (c1 - 1) // 128))
        spans.append((sp, (n, sl)))

    assign = _assign_slots(spans)
    sched = []
    for s in range(SPC):
        u = np.zeros((NG, NT), bool)
        for i in assign[s]:
            for (g, t0, t1) in spans[i][0]:
                u[g, t0 : t1 + 1] = True
        sched.append(tuple(tuple(np.nonzero(u[g])[0].tolist()) for g in range(NG)))
    sched = tuple(sched)

    # [B, S] -> [B, 128, NT]: per-partition scalar columns per char tile
    widT = wid.reshape(B, NT, 128).transpose(0, 2, 1)
    # 1/len per word -> [B, 128, NG] per-partition scalars per word group
    wrecwT = wrecw.reshape(B, NG, 128).transpose(0, 2, 1)

    feats16 = feats.astype(np.float16)
    tab16 = pos_table.astype(np.float16)

    in_maps = []
    for c in range(N_CORES):
        cpb = np.zeros((64, CPB_W), np.float16)
        cpb[0:NPOS, CPB_TAB : CPB_TAB + D] = tab16
        cpr = np.zeros((128, CPR_W), np.float32)
        feats_c = np.empty((SPC, 128, NT * D), np.float16)
        for s in range(SPC):
            i = assign[s][c]
            # char c at partition c%128, cols (c//128)*D : (c//128+1)*D
            feats_c[s] = (
                feats16[i].reshape(NT, 128, D).transpose(1, 0, 2).reshape(128, NT * D)
            )
            cpr[:, CPR_META + NT * s : CPR_META + NT * (s + 1)] = widT[i]
            cpr[:, CPR_REC + NG * s : CPR_REC + NG * (s + 1)] = wrecwT[i]
            cpb[0:NPOS, CPB_POH + s * W : CPB_POH + (s + 1) * W] = (
                posoh[i] * lenw[i][None, :]
            ).astype(np.float16)
        in_maps.append({"feats": feats_c, "constpb": cpb, "constpr": cpr})
    return in_maps, sched, assign


def _run(in_maps, sched, assign, trace=False):
    from concourse.bass_utils import run_bass_kernel_spmd

    nc = _get_program(sched)
    res = run_bass_kernel_spmd(nc, in_maps, list(range(N_CORES)), trace=trace)
    out = np.empty((B, W, D), np.float32)
    for c in range(N_CORES):
        for s in range(SPC):
            # un-swizzle: word w at partition w%128, col block w//128
            o = res.results[c]["out"][s]  # [128, NG*D] fp16
            out[assign[s][c]] = (
                o.reshape(128, NG, D).transpose(1, 0, 2).reshape(W, D)
            ).astype(np.float32)
    return out, res


def kernel(feats, word_lens, seq_len, pos, pos_table):
    in_maps, sched, assign = _prep_inputs(feats, word_lens, seq_len, pos, pos_table)
    out, _ = _run(in_maps, sched, assign, trace=False)
    return out


# revision 18
# speedup vs baseline: 1.1625x; 1.0560x over previous
"""Trainium2 Bass kernel for CharModel ragged segment-mean + pos embedding.

Computation (per sample):
  out[j, :] = mean(feats[start_j:end_j, :]) * valid_j + pos_table[pos_j]
where the ragged segments are given by sorted word start offsets.

Strategy (fp16 data path; harness gate is rel_err < 2e-2 and fp16 lands
at ~7e-4, so no hi/lo compensation is needed):
  - Host precomputes per-char metadata: word_id[c] (which word each char
    belongs to, -1 for padding chars) and wrec[c] = 1/len(word(c)).
  - Device builds a one-hot matrix M[c, j] = (word_id[c]==j) with one
    fused DVE tensor_scalar op per 128-char tile, then the PE computes
    sum[W, D] = M.T @ feats directly in PSUM.  The pos embedding is
    added by accumulating (onehot(pos)*len).T @ pos_table into the same
    PSUM (both operands zero-padded to 128 contraction rows so the PE
    never switches tile quadrants), so the final 1/len multiply leaves
    mean + pos_table[pos].
  - feats ship pre-transposed in DRAM as [128, NT*D] fp16 (char c lives
    at partition c%128, tile c//128) so each per-sample feats DMA is
    per-partition contiguous: 128 descriptors instead of 128*NT, which
    cuts the HWDGE DIRECT2D issue cost ~4x.  The output is staged and
    written the same way ([128, NG*D] fp16, partition-contiguous) and
    un-swizzled on the host.
  - All DMAs ride the sync (HWDGE) queue with the metadata packs FIRST:
    SDMA rings drain in enqueue order, so const-after-feats would stall
    the warm-up + one-hot builds ~6us behind sample-0 feats data.
  - Data parallel over batch: 8 NeuronCores x 4 samples each, one shared
    SPMD program, per-core input maps.

Walrus ISA wait-slot limits dealt with throughout: matmul (S3_LW),
tensor_scalar (S3D3_TS) and DMA (PSEUDO_DMA_DIRECT2D) instructions can
carry only ONE semaphore wait each.  Hence:
  - the fp16 metadata (iota row + word-id scalars) ships in ONE packed
    tensor; the fp32 copy-scale reciprocals in a tiny second pack that a
    Scalar probe and a Vector probe observe once, so the PSUM->SBUF
    copies carry only their PE wait;
  - a per-sample 1x1 "gate" matmul (forced first in PE order via
    add_dep_helper, writing a dedicated never-overlapping PSUM region)
    alone carries the DVE wait for the lhsT builds and, transitively, the
    previous sample's PSUM bank releases;
  - feats tiles and output staging tiles get enough pool bufs that no
    slot is ever reused (no WAR waits on DMAs/copies);
  - two output DMAs per sample (SWDGE, each preceded by a Pool probe of
    the DVE-written half so the DMA carries only the ACT copy wait).
"""

import sys

if "/opt/trn_rl_repo" not in sys.path:
    sys.path.insert(0, "/opt/trn_rl_repo")

import numpy as np

import bass_rust
import concourse.bass as bass
import concourse.mybir as mybir
from concourse.tile import TileContext
from concourse.tile_sem_assignment import N_PROCS


class ChunkedDrainTileContext(TileContext):
    """TileContext whose kernel-tail drain is split into several drain
    instructions with one sem wait each (the CTRL_NO ISA struct rejects
    multi-wait drains here)."""

    DRAIN_CHUNK = 1

    def _drain_and_barrier(self, tick_clock, wait_clock):
        gc = tick_clock.global_clock
        ticks = [gc.peek_next(i) - 1 for i in range(N_PROCS)]
        active = [i for i, t in enumerate(ticks) if t > 0]
        for i in range(0, len(active), self.DRAIN_CHUNK):
            chunk = set(active[i : i + self.DRAIN_CHUNK])
            part = [ticks[j] if j in chunk else 0 for j in range(N_PROCS)]
            d = self.nc.sync.drain()
            wait_clock.add_sem_waits(
                d.ins, bass_rust.ScopedClock({None: bass_rust.VectorClock(part)})
            )
        self.nc.all_engine_barrier()
        assert self.sems is not None
        popped = self.nc._tile_sem_poison_stack.pop()
        assert popped is self._sem_poison
        self.nc.clear_and_free_semaphores(list(self.sems.allocated().values()))
        self.nc.all_engine_barrier()

B, S, D, W, NPOS = 32, 1024, 512 + 256, 512, 32  # D=768
N_CORES = 8
SPC = B // N_CORES  # samples per core
NT = S // 128  # char tiles per sample
NG = W // 128  # word groups per sample
CHUNKS = ((0, 384), (384, 384))  # D split for PSUM bank limit
F32 = mybir.dt.float32
F16 = mybir.dt.float16

# fp16 pos pack: only the 32 live rows ship over DMA; the zero rows
# 32:128 are memset on the DVE (shipping them would double the pack's
# critical-path DMA time)
CPB_POH = 0  # [32, SPC*W]: sample s cols s*W:(s+1)*W = onehot(pos)*len
CPB_TAB = SPC * W  # [32, D]: fp16(pos_table)
CPB_W = SPC * W + D

# fp32 pack (copy scales + word-id scalars; the is_equal build with f32
# stream + f32 scalar + fp16 out is the HW-proven config).  The iota row
# is generated on-device by the Pool engine, keeping this pack tiny so
# the one-hot builds start as early as possible.
CPR_REC = 0  # [128, NG*SPC]: per sample s: 1/len per word group
CPR_META = NG * SPC  # [128, NT*SPC]: per sample s: word-id per char
CPR_W = NG * SPC + NT * SPC


def _build_program(sched):
    """sched[s][g] = tuple of char-tile indices whose chars can touch word
    group g of slot-s samples on ANY core (union schedule; the one-hot
    lhsT zeroes contributions from tiles/words not actually present on a
    given core).  Matmuls for (g, t) pairs outside the schedule multiply
    all-zero one-hot slices and are skipped entirely."""
    nc = bass.Bass()
    # feats pre-transposed on host: char c of sample s lives at
    # [s, c % 128, (c // 128) * D : (c // 128 + 1) * D].
    feats = nc.declare_dram_parameter("feats", [SPC, 128, NT * D], F16, False)
    constpb = nc.declare_dram_parameter("constpb", [64, CPB_W], F16, False)
    constpr = nc.declare_dram_parameter("constpr", [128, CPR_W], F32, False)
    # output staged the same way: word w of sample s at
    # [s, w % 128, (w // 128) * D : ...]; host un-swizzles.
    out = nc.declare_dram_parameter("out", [SPC, 128, NG * D], F16, True)

    dep = lambda a, b, why: bass_rust.add_dep_helper(
        a.ins, b.ins, sync=False, reason=why
    )

    # per-sample used tile range (contiguous from 0 by construction)
    tmax = [max(t for g in range(NG) for t in sched[s][g]) for s in range(SPC)]
    with ChunkedDrainTileContext(nc) as tc:
        with (
            tc.tile_pool(name="const", bufs=1) as cpool,
            tc.tile_pool(name="feat", bufs=SPC) as fpool,
            tc.tile_pool(name="lhs", bufs=sum(t + 1 for t in tmax)) as lpool,
            tc.tile_pool(name="outsb", bufs=SPC) as opool,
            tc.tile_pool(name="psum", bufs=2 * NG, space="PSUM") as ppool,
        ):
            cpb = cpool.tile([128, CPB_W], F16)
            # iota row 0..W-1 on every partition, generated on-device
            # (fp32 is exact for these integers)
            iota_t = cpool.tile([128, W], F32)
            nc.gpsimd.iota(
                iota_t[:, :],
                pattern=[[1, W]],
                base=0,
                channel_multiplier=0,
                allow_small_or_imprecise_dtypes=True,
            )
            iota_f = iota_t[:, :]
            cpr = cpool.tile([128, CPR_W], F32)
            cpr_dma = nc.sync.dma_start(out=cpr[:, :], in_=constpr[:, :])
            # ACT probe: observe the f32-pack DMA on the Scalar engine once,
            # so the ACT PSUM->SBUF copies carry only their PE wait.  (The
            # DVE observes it via the first one-hot build.)  The DVE probe
            # observes the Pool iota tick so the builds carry only the
            # f32-pack wait.
            act_probe = cpool.tile([1, 2], F32)
            nc.scalar.copy(act_probe[0:1, 0:1], cpr[0:1, 0:1])
            nc.vector.tensor_copy(act_probe[0:1, 1:2], iota_t[0:1, 0:1])
            pl_probe = cpool.tile([1, 2 * SPC], F16)
            # cpb rows 64:128 zeroed on the Pool engine AFTER the iota (the
            # DMA ships rows 0:64, with 32:64 zero from the host); a tiny
            # "pgate" matmul below carries the memset tick for the pos
            # matmuls.
            nc.gpsimd.memset(cpb[64:128, :], 0.0)
            # Warm-up bank comes from the SAME pool as the matmul units:
            # sample 0's unit 7 recycles it with a free PE-order release.
            # All gates also write into pool tiles (their outputs are never
            # read; the units' start=True matmuls reset the banks), so all
            # 8 PSUM banks serve the matmul pipeline.
            warm = ppool.tile([128, 384], F32, tag="ps", name="warm")
            # PE warm-up: fat fp32 matmuls reading the on-device iota tile
            # run during the DMA ramp and trip the HAM clock gate before
            # the real matmuls start.  Without this the PE sometimes stays
            # at 1.2GHz for the whole kernel.
            for wi in range(9):
                nc.tensor.matmul(
                    warm[0:1, 0:320],
                    iota_t[:, 0:1],
                    iota_t[:, 0:320],
                    start=(wi == 0),
                    stop=(wi == 8),
                    skip_group_check=True,
                )
            # pgate: carries the Pool memset tick; every pos matmul deps on
            # it so none carries a second wait besides the cpb DMA sem.
            pgate = nc.tensor.matmul(
                warm[0:1, 0:1],
                cpb[64:65, 0:1],
                cpb[64:65, 0:1],
                start=True,
                stop=True,
                skip_group_check=True,
            )
            # feats DMAs: sample 0 in two pieces (so group-0/1 matmuls can
            # start as soon as their tiles land), later samples whole:
            # 7 sync-queue DMAs total, within the 8 HWDGE queues.
            fts = []
            for s in range(SPC):
                ft = fpool.tile([128, NT * D], F16, tag="ft", name=f"ft_{s}")
                fts.append(ft)
            # Input phasing (HWDGE queues bandwidth-share, so ordering
            # only comes from completion-sem deps; each hop costs ~1.4us
            # of issue+straggler, so the chain keeps ~2 transfers in
            # flight and sizes pieces to land just-in-time): cpr and the
            # group-0/1 tiles of sample 0 stream first, everything else
            # chains pairwise behind them.
            ntile0 = tmax[0] + 1
            c1 = min(max(max(sched[0][0]), max(sched[0][1])) + 1, ntile0)
            s0a = nc.sync.dma_start(
                out=fts[0][:, : c1 * D], in_=feats[0, :, : c1 * D]
            )
            cpb_dma = nc.sync.dma_start(out=cpb[0:64, :], in_=constpb[:, :])
            bass_rust.add_dep_helper(
                cpb_dma.ins, cpr_dma.ins, reason="pos pack after f32 pack"
            )
            prev_a, prev_b = s0a, None
            if c1 < ntile0:
                s0b = nc.sync.dma_start(
                    out=fts[0][:, c1 * D : ntile0 * D],
                    in_=feats[0, :, c1 * D : ntile0 * D],
                )
                bass_rust.add_dep_helper(
                    s0b.ins, cpb_dma.ins, reason="s0 tail after pos pack"
                )
                prev_b = s0b
            # sample 1 in halves, samples 2/3 whole, chained pairwise
            n1 = tmax[1] + 1
            h1 = (n1 + 1) // 2
            s1a = nc.sync.dma_start(
                out=fts[1][:, : h1 * D], in_=feats[1, :, : h1 * D]
            )
            bass_rust.add_dep_helper(s1a.ins, prev_a.ins, reason="chain")
            s1b = nc.sync.dma_start(
                out=fts[1][:, h1 * D : n1 * D], in_=feats[1, :, h1 * D : n1 * D]
            )
            bass_rust.add_dep_helper(
                s1b.ins, (prev_b or prev_a).ins, reason="chain"
            )
            fd = nc.sync.dma_start(
                out=fts[2][:, : (tmax[2] + 1) * D],
                in_=feats[2, :, : (tmax[2] + 1) * D],
            )
            bass_rust.add_dep_helper(fd.ins, s1a.ins, reason="chain")
            fd = nc.sync.dma_start(
                out=fts[3][:, : (tmax[3] + 1) * D],
                in_=feats[3, :, : (tmax[3] + 1) * D],
            )
            bass_rust.add_dep_helper(fd.ins, s1b.ins, reason="chain")

            def emit_builds(s):
                d = {}
                for t in range(tmax[s] + 1):
                    lh = lpool.tile([128, W], F16, tag="lh", name=f"lh_{s}_{t}")
                    wcol = CPR_META + NT * s
                    nc.vector.tensor_scalar(
                        lh[:, :],
                        iota_f,
                        cpr[:, wcol + t : wcol + t + 1],
                        None,
                        op0=mybir.AluOpType.is_equal,
                    )
                    d[t] = lh
                return d

            lhs_by_s = {0: emit_builds(0)}
            prev_ob = None  # previous sample's output staging buffer
            for s in range(SPC):
                ft = fts[s]
                last = s == SPC - 1
                last_dve_copy = None
                last_act_copy = None
                ntile = tmax[s] + 1
                lhs = lhs_by_s[s]
                gate = gate2 = None
                bank_gates = []
                ob = opool.tile([128, NG * D], F16, tag="ob", name=f"ob_{s}")
                for g in range(NG):
                    if g == 2 and s + 1 < SPC:
                        # Emit the NEXT sample's one-hot builds here: they
                        # slot into the DVE queue between this sample's
                        # unit-2 and unit-4 copies, so they never sit behind
                        # a copy that is blocked on a late PSUM (DVE
                        # head-of-line), and the next gateA finds them done.
                        lhs_by_s[s + 1] = emit_builds(s + 1)
                    tiles_g = sched[s][g]
                    pss = [
                        ppool.tile([128, cn], F32, tag="ps", name=f"ps_{s}_{g}_{ci}")
                        for ci, (c0, cn) in enumerate(CHUNKS)
                    ]
                    # Bank gates at g0 and g2: each reads a prev-sample ob
                    # cell whose copy tick (same engine, same or later unit)
                    # also covers the release of the bank it writes, so one
                    # sem wait per gate -- and that tick is >= one group old
                    # by the time the PE gets here, so the PE never stalls
                    # at sample boundaries (stalls reset the HAM clock ramp
                    # and cost ~2x the gap).
                    if prev_ob is not None and g in (0, 2):
                        dcell = D if g == 0 else 3 * D
                        acell = D + 384 if g == 0 else NG * D - 1
                        gd = nc.tensor.matmul(
                            pss[0][0:1, 0:1],
                            prev_ob[0:1, dcell : dcell + 1],
                            prev_ob[0:1, dcell : dcell + 1],
                            start=True,
                            stop=True,
                            skip_group_check=True,
                        )
                        ga = nc.tensor.matmul(
                            pss[1][0:1, 0:1],
                            prev_ob[0:1, acell : acell + 1],
                            prev_ob[0:1, acell : acell + 1],
                            start=True,
                            stop=True,
                            skip_group_check=True,
                        )
                        bank_gates = [gd, ga]
                    if g == 0:
                        # build gate: carries the DVE wait for this sample's
                        # one-hot builds (group 0's tiles only for the
                        # head-critical sample 0)
                        tA = (
                            max(max(sched[s][0]), max(sched[s][1]))
                            if s == 0
                            else ntile - 1
                        )
                        gate = nc.tensor.matmul(
                            pss[0][0:1, 1:2],
                            lhs[tA][0:1, 0:1],
                            lhs[tA][0:1, 0:1],
                            start=True,
                            stop=True,
                            skip_group_check=True,
                        )
                    if g == 2 and s == 0:
                        gate2 = nc.tensor.matmul(
                            pss[0][0:1, 1:2],
                            lhs[ntile - 1][0:1, 0:1],
                            lhs[ntile - 1][0:1, 0:1],
                            start=True,
                            stop=True,
                            skip_group_check=True,
                        )

                    def _mm_deps(mm):
                        dep(mm, gate, "matmuls after sample build gate")
                        if gate2 is not None and g >= 2:
                            dep(mm, gate2, "matmuls after late build gate")
                        for bg in bank_gates:
                            dep(mm, bg, "matmuls after bank gate")

                    # chunk-inner order: the two matmuls of a (g, t) pair use
                    # the same stationary weights back to back
                    for k, t in enumerate(tiles_g):
                        for ci, (c0, cn) in enumerate(CHUNKS):
                            mm = nc.tensor.matmul(
                                pss[ci][:, :],
                                lhs[t][:, 128 * g : 128 * (g + 1)],
                                ft[:, t * D + c0 : t * D + c0 + cn],
                                start=(k == 0),
                                stop=False,
                                skip_group_check=True,
                            )
                            _mm_deps(mm)
                    # pos contribution scaled by len (exact in fp16) so the
                    # final 1/len multiply leaves pos_table[pos] exactly;
                    # both operands are zero-padded to 128 rows
                    pcol = CPB_POH + s * W
                    for ci, (c0, cn) in enumerate(CHUNKS):
                        mm = nc.tensor.matmul(
                            pss[ci][:, :],
                            cpb[:, pcol + 128 * g : pcol + 128 * (g + 1)],
                            cpb[:, CPB_TAB + c0 : CPB_TAB + c0 + cn],
                            start=(len(tiles_g) == 0),
                            stop=True,
                            skip_group_check=True,
                        )
                        dep(mm, pgate, "pos matmul after memset gate")
                        _mm_deps(mm)
                    for ci, (c0, cn) in enumerate(CHUNKS):
                        unit = 2 * g + ci
                        recip_ap = cpr[
                            :, CPR_REC + NG * s + g : CPR_REC + NG * s + g + 1
                        ]
                        # last sample: units 0-3 all on ACT so the g1-point
                        # output DMA needs only the chained unit-3 ACT wait
                        use_dve = (unit % 2 == 0) and not (last and unit < 4)
                        if use_dve:
                            cp = nc.vector.tensor_scalar(
                                ob[:, g * D + c0 : g * D + c0 + cn],
                                pss[ci][:, :],
                                recip_ap,
                                None,
                                op0=mybir.AluOpType.mult,
                            )
                            if last_dve_copy is not None:
                                dep(cp, last_dve_copy, "DVE copy order")
                            last_dve_copy = cp
                        else:
                            cp = nc.scalar.activation(
                                ob[:, g * D + c0 : g * D + c0 + cn],
                                pss[ci][:, :],
                                mybir.ActivationFunctionType.Copy,
                                scale=recip_ap,
                            )
                            if last_act_copy is not None:
                                dep(cp, last_act_copy, "ACT copy order")
                            last_act_copy = cp
                    # streaming output: sample 0 ships once at the end
                    # (it is never tail-critical); middle samples split at
                    # the g2 point; the last sample ships three pieces so
                    # most of its output flows during its own compute.
                    # Pool probes observe the DVE copy chain so each DMA
                    # carries only the chained ACT wait.
                    if last and g == 1:
                        # units 0-3 are all ACT copies: the DMA's unit-3
                        # wait covers them by the ACT dep chain
                        nc.gpsimd.dma_start(
                            out=out[s, :, : 2 * D], in_=ob[:, : 2 * D]
                        )
                    if g == 2 and s != 0:
                        lo = 2 * D if last else 0
                        nc.gpsimd.tensor_copy(
                            pl_probe[0:1, 2 * s : 2 * s + 1],
                            ob[0:1, 2 * D : 2 * D + 1],
                        )
                        nc.gpsimd.dma_start(
                            out=out[s, :, lo : 3 * D], in_=ob[:, lo : 3 * D]
                        )
                # Pool probe: observe the last DVE copy's tick on the Pool
                # engine so the output DMA carries only the ACT copy wait.
                nc.gpsimd.tensor_copy(
                    pl_probe[0:1, 2 * s + 1 : 2 * s + 2], ob[0:1, 3 * D : 3 * D + 1]
                )
                lo = 3 * D if s != 0 else 0
                nc.gpsimd.dma_start(out=out[s, :, lo:], in_=ob[:, lo:])
                prev_ob = ob
    return nc


_PROGRAM_CACHE = {}


def _get_program(sched):
    key = tuple(tuple(tuple(g) for g in s) for s in sched)
    if key not in _PROGRAM_CACHE:
        _PROGRAM_CACHE[key] = _build_program(sched)
    return _PROGRAM_CACHE[key]


def _assign_slots(spans):
    """Assign the B samples to (slot, core) so that the per-slot UNION of
    (group, char-tile) matmul footprints is small: sort by profile, then
    cheap local-search swaps."""
    import random

    def union_cost(assign):
        total = 0
        for slot in assign:
            u = np.zeros((NG, NT), bool)
            for i in slot:
                for (g, t0, t1) in spans[i][0]:
                    u[g, t0 : t1 + 1] = True
            total += int(u.sum())
        return total

    order = sorted(range(B), key=lambda i: spans[i][1])
    best_assign, best = None, None
    for seed in range(6):
        rng = random.Random(seed)
        assign = [
            [order[s * N_CORES + c] for c in range(N_CORES)] for s in range(SPC)
        ]
        if seed:
            flat = [i for slot in assign for i in slot]
            rng.shuffle(flat)
            assign = [flat[s * N_CORES : (s + 1) * N_CORES] for s in range(SPC)]
        cost = union_cost(assign)
        for _ in range(30000):
            s1, s2 = rng.randrange(SPC), rng.randrange(SPC)
            if s1 == s2:
                continue
            i1, i2 = rng.randrange(N_CORES), rng.randrange(N_CORES)
            assign[s1][i1], assign[s2][i2] = assign[s2][i2], assign[s1][i1]
            c = union_cost(assign)
            if c <= cost:
                cost = c
            else:
                assign[s1][i1], assign[s2][i2] = assign[s2][i2], assign[s1][i1]
        if best is None or cost < best:
            best, best_assign = cost, [list(s) for s in assign]
    return best_assign


def _prep_inputs(feats, word_lens, seq_len, pos, pos_table):
    """Host-side metadata prep + batch sharding -> per-core input maps,
    union matmul schedule, and the sample->(slot, core) assignment."""
    feats = np.ascontiguousarray(np.asarray(feats), dtype=np.float32)
    word_lens = np.asarray(word_lens).astype(np.int64)
    seq_len = np.asarray(seq_len).astype(np.int64)
    pos = np.asarray(pos).astype(np.int64)
    pos_table = np.ascontiguousarray(np.asarray(pos_table), dtype=np.float32)

    wid = np.full((B, S), -1.0, np.float32)
    wrecw = np.zeros((B, W), np.float32)  # 1/len per word (0 for padding)
    lenw = np.zeros((B, W), np.float32)  # len per word (0 for padding)
    posoh = np.zeros((B, NPOS, W), np.float32)
    spans = []  # per sample: ([(g, t0, t1), ...], profile_key)
    for i in range(B):
        wl = word_lens[i]
        sl = int(seq_len[i])
        valid = wl != 0
        valid[0] = True
        ridx = np.nonzero(valid)[0]  # real words (contiguous prefix by construction)
        starts = wl[ridx]
        n = len(ridx)
        nxt = np.append(starts[1:], 0)
        ends = np.where(nxt == 0, sl, nxt)
        lens = np.maximum(ends - starts, 1)
        cidx = np.arange(sl)
        cwid = np.searchsorted(starts, cidx, side="right") - 1
        wid[i, :sl] = ridx[cwid].astype(np.float32)
        wrecw[i, ridx] = 1.0 / lens.astype(np.float32)
        lenw[i, ridx] = lens.astype(np.float32)
        posoh[i, pos[i], np.arange(W)] = 1.0  # one-hot part
        sp = []
        for g in range(NG):
            w0 = 128 * g
            if w0 >= n:
                continue
            w1 = min(128 * (g + 1), n)
            c0, c1 = int(starts[w0]), int(ends[w1 - 1])
            sp.append((g, c0 // 128, (c1 - 1) // 128))
        spans.append((sp, (n, sl)))

    assign = _assign_slots(spans)
    sched = []
    for s in range(SPC):
        u = np.zeros((NG, NT), bool)
        for i in assign[s]:
            for (g, t0, t1) in spans[i][0]:
                u[g, t0 : t1 + 1] = True
        sched.append(tuple(tuple(np.nonzero(u[g])[0].tolist()) for g in range(NG)))
    sched = tuple(sched)

    # [B, S] -> [B, 128, NT]: per-partition scalar columns per char tile
    widT = wid.reshape(B, NT, 128).transpose(0, 2, 1)
    # 1/len per word -> [B, 128, NG] per-partition scalars per word group
    wrecwT = wrecw.reshape(B, NG, 128).transpose(0, 2, 1)

    feats16 = feats.astype(np.float16)
    tab16 = pos_table.astype(np.float16)

    in_maps = []
    for c in range(N_CORES):
        cpb = np.zeros((64, CPB_W), np.float16)
        cpb[0:NPOS, CPB_TAB : CPB_TAB + D] = tab16
        cpr = np.zeros((128, CPR_W), np.float32)
        feats_c = np.empty((SPC, 128, NT * D), np.float16)
        for s in range(SPC):
            i = assign[s][c]
            # char c at partition c%128, cols (c//128)*D : (c//128+1)*D
            feats_c[s] = (
                feats16[i].reshape(NT, 128, D).transpose(1, 0, 2).reshape(128, NT * D)
            )
            cpr[:, CPR_META + NT * s : CPR_META + NT * (s + 1)] = widT[i]
            cpr[:, CPR_REC + NG * s : CPR_REC + NG * (s + 1)] = wrecwT[i]
            cpb[0:NPOS, CPB_POH + s * W : CPB_POH + (s + 1) * W] = (
                posoh[i] * lenw[i][None, :]
            ).astype(np.float16)
        in_maps.append({"feats": feats_c, "constpb": cpb, "constpr": cpr})
    return in_maps, sched, assign


def _run(in_maps, sched, assign, trace=False):
    from concourse.bass_utils import run_bass_kernel_spmd

    nc = _get_program(sched)
    res = run_bass_kernel_spmd(nc, in_maps, list(range(N_CORES)), trace=trace)
    out = np.empty((B, W, D), np.float32)
    for c in range(N_CORES):
        for s in range(SPC):
            # un-swizzle: word w at partition w%128, col block w//128
            o = res.results[c]["out"][s]  # [128, NG*D] fp16
            out[assign[s][c]] = (
                o.reshape(128, NG, D).transpose(1, 0, 2).reshape(W, D)
            ).astype(np.float32)
    return out, res


def kernel(feats, word_lens, seq_len, pos, pos_table):
    in_maps, sched, assign = _prep_inputs(feats, word_lens, seq_len, pos, pos_table)
    out, _ = _run(in_maps, sched, assign, trace=False)
    return out
